# revision 10
# baseline (speedup 1.0000x reference)
"""2-layer GCN encoder as a distributed Bass kernel on 8 TRN2 NeuronCores.

Decomposition (per core, nodes sharded by destination):
  hs1[v]  = dinv[v] * (x[v] @ W1)                 (own rows, bf16)
  S1T[:,d]= sum_{e: dst=d} hs1[src_e]             (dma_gather + one-hot matmul,
                                                   accumulated TRANSPOSED)
  hsrT    = relu(dinv_col * S1T + b1)             (dst-side dinv per column)
  hsr2[v] = dinv[v] * (hsrT^T @ W2)               (W2 commutes with the layer-2
                                                   edge sum -> aggregate at 64)
  S2[d]   = sum_{e: dst=d} hsr2[src_e]
  y[d]    = dinv[d]*S2[d] + b2

The critical resource is SWDGE descriptor emission (~2ns of Q7 time per
gathered row, 4 queue-pairs).  Sources are split into three streams per
destination block:
  own  - sources owned by this core, gathered from the LOCAL stage tensor;
         no collective dependency, so these gathers run inside the CC-barrier
         / AllGather windows that would otherwise idle the Q7.
  g0   - remote sources from every rank's first GSPLIT blocks (AllGather 0)
  g1   - remote sources from the remaining blocks (AllGather 1)
Each layer's AllGather runs as two chunked collectives; layer-2's triggers
are placed after ALL of layer 1's gather instructions so they never stall
the in-order GpSimd stream.  Remote segments issue with g0 lookahead
([g0 r0][g0 r1][g0 r2][g1 r0][g0 r3][g1 r1]...) so g1 consumption starts
only after its AllGather has had time to complete.

Destination blocks are processed in rounds of 6 (PSUM allocations are
bank-granular: 6 accumulators + 2 aux = 8 banks).  Gather instructions pack
up to MAXCH=16 chunks across block boundaries (single_packet=False is
required above 64 descriptors per engine).  Per-segment valid counts are
uniform across cores (num_idxs_reg); interior pads gather row 0 against
zero one-hot columns, trailing pads are -1 on every core identically.
"""

import numpy as np

import concourse.bass as bass
import concourse.bacc as bacc
import concourse.mybir as mybir
import concourse.tile as tile
from concourse import library_config
from concourse.bass_utils import run_bass_kernel_spmd

F32 = mybir.dt.float32
BF16 = mybir.dt.bfloat16
FP8 = mybir.dt.float8e4
I16 = mybir.dt.int16

NCORES = 8
BLK = 128
MAXCH = 16     # chunks (128 idx each) per dma_gather instruction
NQUEUES = 4
NS = 3         # source streams: 0,1 remote AllGather groups; 2 = own-core


def _cdiv(a, b):
    return (a + b - 1) // b


def preprocess(x, edge_index, ncores=NCORES):
    """Host-side graph partitioning: shard nodes/edges by dst, split sources
    into own/remote-g0/remote-g1 streams, build per-core gather indices and
    the one-hot chunk matrices (fp8)."""
    import ml_dtypes

    N, IN = x.shape
    assert N % ncores == 0
    NP = N // ncores
    nblk = _cdiv(NP, BLK)
    R = 6                              # blocks per processing round
    NR = _cdiv(nblk, R)
    GSPLIT = 12                        # remote source-group boundary (blocks)
    gb = [0, GSPLIT * BLK, NP]
    NPg = [gb[1] - gb[0], gb[2] - gb[1]]
    widths = [min(BLK, NP - b * BLK) for b in range(nblk)]
    rounds = [list(range(r * R, min((r + 1) * R, nblk))) for r in range(NR)]

    src = np.asarray(edge_index[0], dtype=np.int64)
    dst = np.asarray(edge_index[1], dtype=np.int64)
    deg = (np.bincount(dst, minlength=N) + 1).astype(np.float32)

    # dedupe repeated (src, dst) pairs; multiplicity goes into the multi-hot
    key = dst * N + src
    ukey, mult = np.unique(key, return_counts=True)
    dst_s = ukey // N
    src_s = ukey % N
    mult = mult.astype(np.float32)

    srcr = src_s // NP
    srco = src_s % NP
    sgrem = (srco >= gb[1]).astype(np.int64)         # remote group id
    growrem = srcr * np.asarray(NPg)[sgrem] + (srco - np.asarray(gb)[sgrem])

    bounds = np.array(
        [i * NP + b * BLK for i in range(ncores) for b in range(nblk)] + [N],
        dtype=np.int64,
    )
    pos = np.searchsorted(dst_s, bounds)

    # per (core, block, stream): sorted unique rows + scatter triplets
    blk_rows = {}
    blk_scatter = {}
    ucnt = np.zeros((ncores, nblk, NS), np.int64)
    for i in range(ncores):
        for b in range(nblk):
            k = i * nblk + b
            s0, s1 = pos[k], pos[k + 1]
            dl = (dst_s[s0:s1] - (i * NP + b * BLK)).astype(np.int64)
            own = srcr[s0:s1] == i
            stream = np.where(own, 2, sgrem[s0:s1])
            row = np.where(own, srco[s0:s1], growrem[s0:s1])
            for g in range(NS):
                m = stream == g
                urows, inv = np.unique(row[m], return_inverse=True)
                ucnt[i, b, g] = len(urows)
                blk_rows[i, b, g] = urows
                blk_scatter[i, b, g] = (inv, dl[m], mult[s0:s1][m])

    CH = np.maximum(1, _cdiv(ucnt.max(axis=0), 128)).astype(np.int64)  # [b, g]

    # chunk layout: own stream first (all blocks), then per round g0, g1
    cbase = {}
    c = 0
    for b in range(nblk):
        cbase[b, 2] = c
        c += int(CH[b, 2])
    own_end = c
    stream_span = {}
    for r in range(NR):
        for g in range(2):
            st = c
            for b in rounds[r]:
                cbase[b, g] = c
                c += int(CH[b, g])
            stream_span[r, g] = (st, c)
    NCHT = c

    # segments (stream_kind, c0, sch): own packs across all blocks; remote
    # packs within each (round, g) stream
    own_segs = [(c0, min(MAXCH, own_end - c0)) for c0 in range(0, own_end, MAXCH)]
    rem_segs = {}
    for r in range(NR):
        for g in range(2):
            st, en = stream_span[r, g]
            rem_segs[r, g] = [(c0, min(MAXCH, en - c0))
                              for c0 in range(st, en, MAXCH)]
    segments = [("own", c0, sch) for (c0, sch) in own_segs]
    for r in range(NR):
        for g in range(2):
            segments += [(g, c0, sch) for (c0, sch) in rem_segs[r, g]]

    # first pass: raw index values per core (-1 where no real source)
    all_vals = []
    all_ohs = []
    for i in range(ncores):
        vals = np.full(NCHT * 128, -1, np.int64)
        ohs = np.zeros((128, NCHT * 128), np.float32)
        for b in range(nblk):
            for g in range(NS):
                urows = blk_rows[i, b, g]
                inv, dl, mlt = blk_scatter[i, b, g]
                c0 = cbase[b, g]
                vals[c0 * 128 : c0 * 128 + len(urows)] = urows
                gc = c0 + inv // 128
                p = inv % 128
                np.add.at(ohs, (p, gc * 128 + dl), mlt)
        all_vals.append(vals)
        all_ohs.append(ohs)

    # per-segment valid count must be uniform across cores: num_idxs_reg is
    # baked into the shared program, and the ucode's trailing trim must land
    # exactly at the register value on every core
    seg_valid = []
    for (_, c0, sch) in segments:
        nv = 1
        for i in range(ncores):
            v = all_vals[i][c0 * 128 : (c0 + sch) * 128]
            nz = np.nonzero(v >= 0)[0]
            if len(nz):
                nv = max(nv, int(nz[-1]) + 1)
        seg_valid.append(nv)

    per_core = []
    for i in range(ncores):
        vals = all_vals[i]
        gidx = np.zeros((128, NCHT * 8), np.int16)
        for si, (_, c0, sch) in enumerate(segments):
            v = vals[c0 * 128 : (c0 + sch) * 128].copy()
            nv = seg_valid[si]
            v[:nv][v[:nv] < 0] = 0     # interior pads gather row 0
            v[nv:] = -1                # uniform trailing trim point
            wr = v.reshape(sch * 8, 16).T
            gidx[:, c0 * 8 : (c0 + sch) * 8] = np.tile(wr.astype(np.int16), (8, 1))

        degp = np.concatenate(
            [deg[i * NP : (i + 1) * NP], np.ones(nblk * BLK - NP, np.float32)]
        )
        per_core.append(
            {
                "x_tr": np.ascontiguousarray(
                    x[i * NP : (i + 1) * NP].T.astype(ml_dtypes.bfloat16)
                ),
                "deg_own": np.ascontiguousarray(degp.reshape(nblk, BLK).T),
                "deg_row": np.ascontiguousarray(degp.reshape(1, nblk * BLK)),
                "gidx": gidx,
                "ohs": all_ohs[i].astype(ml_dtypes.float8_e4m3),
            }
        )

    meta = {
        "N": N,
        "NP": NP,
        "IN": IN,
        "nblk": nblk,
        "R": R,
        "NR": NR,
        "GSPLIT": GSPLIT,
        "NPg": NPg,
        "rounds": rounds,
        "widths": widths,
        "CH": CH,
        "cbase": cbase,
        "own_segs": own_segs,
        "rem_segs": rem_segs,
        "segments": segments,
        "seg_valid": seg_valid,
        "NCHT": NCHT,
    }
    return per_core, meta


def build_nc(meta, HID, OUT, ncores=NCORES):
    N, NP, IN = meta["N"], meta["NP"], meta["IN"]
    nblk, widths = meta["nblk"], meta["widths"]
    R, NR, NPg, rounds = meta["R"], meta["NR"], meta["NPg"], meta["rounds"]
    GSPLIT = meta["GSPLIT"]
    CH, cbase, NCHT = meta["CH"], meta["cbase"], meta["NCHT"]
    segments, seg_valid = meta["segments"], meta["seg_valid"]
    own_segs, rem_segs = meta["own_segs"], meta["rem_segs"]
    KC = IN // 128
    assert IN % 128 == 0 and HID == 128 and OUT <= 128

    nc = bacc.Bacc(
        "TRN2",
        target_bir_lowering=False,
        debug=False,
        num_devices=ncores,
        num_swdge_queues=NQUEUES,
    )

    x_tr = nc.dram_tensor("x_tr", [IN, NP], BF16, kind="ExternalInput")
    w1 = nc.dram_tensor("w1", [IN, HID], BF16, kind="ExternalInput")
    b1c = nc.dram_tensor("b1c", [HID, 1], F32, kind="ExternalInput")
    w2 = nc.dram_tensor("w2", [HID, OUT], F32, kind="ExternalInput")
    b2 = nc.dram_tensor("b2", [1, OUT], F32, kind="ExternalInput")
    deg_own = nc.dram_tensor("deg_own", [128, nblk], F32, kind="ExternalInput")
    deg_row = nc.dram_tensor("deg_row", [1, nblk * BLK], F32, kind="ExternalInput")
    gidx_d = nc.dram_tensor("gidx", [128, NCHT * 8], I16, kind="ExternalInput")
    ident_d = nc.dram_tensor("ident", [128, 128], BF16, kind="ExternalInput")
    ohs_d = nc.dram_tensor("ohs", [128, NCHT * 128], FP8, kind="ExternalInput")
    y = nc.dram_tensor("y", [NP, OUT], F32, kind="ExternalOutput")

    hs1_stage = nc.dram_tensor("hs1_stage", [NP, HID], BF16)
    hsr2_stage = nc.dram_tensor("hsr2_stage", [NP, 128], BF16)
    hs1_full = [
        nc.dram_tensor(f"hs1_full{g}", [ncores * NPg[g], HID], BF16,
                       addr_space="Shared")
        for g in range(2)
    ]
    hsr2_full = [
        nc.dram_tensor(f"hsr2_full{g}", [ncores * NPg[g], 128], BF16,
                       addr_space="Shared")
        for g in range(2)
    ]
    rg = [list(range(ncores))]
    qn = [0]

    def next_q():
        q = qn[0]
        qn[0] = (q + 1) % NQUEUES
        return q

    # last chunk of each block (end of its g1 stream) -> stop flag
    lastc = {b: cbase[b, 1] + int(CH[b, 1]) - 1 for b in range(nblk)}
    chunk_blk = {}
    for b in range(nblk):
        for g in range(NS):
            for cc in range(int(CH[b, g])):
                chunk_blk[cbase[b, g] + cc] = b
    segv = {c0: nv for (_, c0, _), nv in zip(segments, seg_valid)}

    # remote segment issue order: g0 lookahead so g1 consumption starts
    # only after its AllGather completes
    seq = [(0, 0), (1, 0), (2, 0), (0, 1), (3, 0), (1, 1), (4, 0), (2, 1),
           (3, 1), (4, 1)]

    def remote_order():
        order = []
        for (r, g) in seq:
            order += [(g, c0, sch) for (c0, sch) in rem_segs[r, g]]
        return order

    XG = 10  # blocks per x-load slice

    with tile.TileContext(nc) as tc:
        with (
            tc.tile_pool(name="const", bufs=1) as constp,
            tc.tile_pool(name="hs", bufs=4) as hsp,
        ):
            nc.gpsimd.load_library(library_config.mlp)

            dinv_sb = constp.tile([128, nblk], F32, tag="dinv")
            b2_bc = constp.tile([128, OUT], F32, tag="b2bc")
            dinv_bc = constp.tile([128, nblk * BLK], F32, tag="dinvbc")
            gidx_sb = constp.tile([128, NCHT * 8], I16, tag="gidx")
            ident_sb = constp.tile([128, 128], BF16, tag="ident")
            w2_sb = constp.tile([HID, OUT], F32, tag="w2")
            b1_sb = constp.tile([HID, 1], F32, tag="b1")
            b2_sb = constp.tile([1, OUT], F32, tag="b2")
            ones_sb = constp.tile([1, 128], F32, tag="ones")
            ohs_sb = constp.tile([128, NCHT * 128], FP8, tag="ohs")
            hs1_t = []

            with tc.tile_pool(name="pre", bufs=1) as prep:
                # ---- loads needed by phase B ----
                xsb = {}
                for k in range(KC):
                    for s in range(0, nblk, XG):
                        cols = sum(widths[s : s + XG])
                        t = prep.tile([128, XG * BLK], BF16, tag=f"x{k}_{s}")
                        nc.sync.dma_start(
                            out=t[:, :cols],
                            in_=x_tr[k * 128 : (k + 1) * 128,
                                     s * BLK : s * BLK + cols],
                        )
                        xsb[k, s] = t
                w1c = []
                for k in range(KC):
                    t = prep.tile([128, HID], BF16, tag=f"w1c{k}")
                    nc.sync.dma_start(out=t[:], in_=w1[k * 128 : (k + 1) * 128, :])
                    w1c.append(t)
                nc.sync.dma_start(out=dinv_sb[:], in_=deg_own[:, :])
                nc.scalar.sqrt(dinv_sb[:], dinv_sb[:])
                nc.vector.reciprocal(dinv_sb[:], dinv_sb[:])

                # ---- phase B + broadcasts (own PSUM scope) ----
                with tc.tile_pool(name="psB", bufs=2, space="PSUM") as psB:
                    for b in range(nblk):
                        w = widths[b]
                        ph = psB.tile([128, HID], F32, tag="acc")
                        for k in range(KC):
                            nc.tensor.matmul(
                                ph[:w, :],
                                lhsT=xsb[k, (b // XG) * XG][
                                    :, (b % XG) * BLK : (b % XG) * BLK + w],
                                rhs=w1c[k][:, :],
                                start=(k == 0),
                                stop=(k == KC - 1),
                            )
                        t = constp.tile([128, HID], BF16, tag=f"hs1_{b}",
                                        name=f"hs1t_{b}")
                        nc.scalar.activation(
                            t[:w, :],
                            ph[:w, :],
                            mybir.ActivationFunctionType.Copy,
                            scale=dinv_sb[:w, b : b + 1],
                        )
                        nc.sync.dma_start(
                            out=hs1_stage[b * BLK : b * BLK + w, :], in_=t[:w, :]
                        )
                        hs1_t.append(t)
                        if b == GSPLIT - 1:
                            nc.gpsimd.collective_compute(
                                "AllGather",
                                mybir.AluOpType.bypass,
                                replica_groups=rg,
                                ins=[hs1_stage[0 : NPg[0], :].opt()],
                                outs=[hs1_full[0][0 : ncores * NPg[0], :].opt()],
                            )
                    nc.gpsimd.collective_compute(
                        "AllGather",
                        mybir.AluOpType.bypass,
                        replica_groups=rg,
                        ins=[hs1_stage[NPg[0] : NP, :].opt()],
                        outs=[hs1_full[1][0 : ncores * NPg[1], :].opt()],
                    )

                    # remaining constants (overlap the AllGathers / barrier)
                    nc.sync.dma_start(out=gidx_sb[:], in_=gidx_d[:, :])
                    nc.sync.dma_start(out=ident_sb[:], in_=ident_d[:, :])
                    nc.sync.dma_start(out=w2_sb[:], in_=w2[:, :])
                    nc.sync.dma_start(out=b1_sb[:], in_=b1c[:, :])
                    nc.sync.dma_start(out=b2_sb[:], in_=b2[:, :])
                    nc.vector.memset(ones_sb[:], 1.0)
                    qcols = _cdiv(NCHT * 128, 4)
                    for qq in range(4):
                        c0q = qq * qcols
                        c1q = min((qq + 1) * qcols, NCHT * 128)
                        nc.sync.dma_start(out=ohs_sb[:, c0q:c1q],
                                          in_=ohs_d[:, c0q:c1q])

                    # broadcast b2 to all partitions via rank-1 matmul
                    pb2 = psB.tile([128, 128], F32, tag="aux")
                    nc.tensor.matmul(pb2[:, :OUT], lhsT=ones_sb[:], rhs=b2_sb[:],
                                     start=True, stop=True)
                    nc.vector.tensor_copy(b2_bc[:], pb2[:, :OUT])

                    # per-column dinv for the transposed layer-1 epilogue
                    deg_rsb = prep.tile([1, nblk * BLK], F32, tag="degrow")
                    nc.sync.dma_start(out=deg_rsb[:], in_=deg_row[:, :])
                    for b in range(nblk):
                        pdv = psB.tile([128, 128], F32, tag="aux")
                        nc.tensor.matmul(
                            pdv[:], lhsT=ones_sb[:],
                            rhs=deg_rsb[:, b * BLK : (b + 1) * BLK],
                            start=True, stop=True,
                        )
                        nc.vector.tensor_copy(
                            dinv_bc[:, b * BLK : (b + 1) * BLK], pdv[:])
                    nc.scalar.sqrt(dinv_bc[:], dinv_bc[:])
                    nc.vector.reciprocal(dinv_bc[:], dinv_bc[:])

            with (
                tc.tile_pool(name="gath", bufs=12) as gathp,
                tc.tile_pool(name="gown", bufs=6) as gownp,
            ):
                # zero both gather pools once: trailing-trimmed lanes expose
                # stale SBUF; first use must not contain NaN-decoding garbage
                for zi in range(12):
                    zt = gathp.tile([128, MAXCH, HID], BF16, tag="g",
                                    name=f"z{zi}")
                    nc.vector.memset(zt[:, :, :], 0.0)
                for zi in range(6):
                    zt = gownp.tile([128, MAXCH, HID], BF16, tag="go",
                                    name=f"zo{zi}")
                    nc.vector.memset(zt[:, :, :], 0.0)

                def issue_gather(pool, tag, table, c0, sch, elem):
                    t = pool.tile([128, MAXCH, HID], BF16, tag=tag,
                                  name=f"gt{c0}")
                    nc.gpsimd.dma_gather(
                        t[:, :sch, :],
                        table.ap(),
                        gidx_sb[:, c0 * 8 : (c0 + sch) * 8],
                        sch * 128,
                        segv[c0],
                        elem,
                        queue_num=next_q(),
                        single_packet=False,
                    )
                    return t

                def one_mm(layer, acc, b, t, cc, gc, stop):
                    w = widths[b]
                    if layer == 1:
                        nc.tensor.matmul(
                            acc[b][:, :w],
                            lhsT=t[:, cc, :],
                            rhs=ohs_sb[:, gc * 128 : gc * 128 + w],
                            start=False,
                            stop=stop,
                        )
                    else:
                        nc.tensor.matmul(
                            acc[b][:w, :OUT],
                            lhsT=ohs_sb[:, gc * 128 : gc * 128 + w],
                            rhs=t[:, cc, :OUT],
                            start=False,
                            stop=stop,
                        )

                def run_layer(layer, stage_t, full_t, elem, acc_shape,
                              acc_pool, selfloop, epilogue):
                    # own-stream gathers first: no collective dependency
                    own_tiles = [
                        (issue_gather(gownp, "go", stage_t, c0, sch, elem),
                         c0, sch)
                        for (c0, sch) in own_segs
                    ]
                    rorder = remote_order()
                    rem_tiles = {}
                    ri = [0]

                    def consume_upto(n):
                        while ri[0] < min(n, len(rorder)):
                            g, c0, sch = rorder[ri[0]]
                            rem_tiles[c0] = issue_gather(
                                gathp, "g", full_t[g], c0, sch, elem)
                            ri[0] += 1

                    # by round r's MMs, segments through round r+1's g0 are
                    # issued (position of (r,g1) + following g0 in seq)
                    marks = []
                    for r in range(NR):
                        pos = seq.index((r, 1)) + 1
                        if pos < len(seq) and seq[pos][1] == 0:
                            pos += 1
                        marks.append(sum(len(rem_segs[rr, gg])
                                         for (rr, gg) in seq[:pos]))

                    for r in range(NR):
                        consume_upto(marks[r])
                        acc_cur = {}
                        for b in rounds[r]:
                            acc_cur[b] = acc_pool.tile(
                                acc_shape, F32, tag=f"acc{b - r * R}",
                                name=f"acc{layer}_{b}",
                            )
                            selfloop(acc_cur, b)
                        lo, hi = rounds[r][0], rounds[r][-1]
                        for (t, c0, sch) in own_tiles:
                            for cc in range(sch):
                                gc = c0 + cc
                                b = chunk_blk[gc]
                                if lo <= b <= hi:
                                    one_mm(layer, acc_cur, b, t, cc, gc, False)
                        for g in range(2):
                            for (c0, sch) in rem_segs[r, g]:
                                t = rem_tiles[c0]
                                for cc in range(sch):
                                    gc = c0 + cc
                                    b = chunk_blk[gc]
                                    one_mm(layer, acc_cur, b, t, cc, gc,
                                           gc == lastc[b])
                        for b in rounds[r]:
                            epilogue(acc_cur, b)
                    consume_upto(len(rorder))

                # ---- layer 1: S1^T -> hsr^T -> hsr2 ----
                hsr2_t = [None] * nblk
                with tc.tile_pool(name="psD", bufs=1, space="PSUM") as psD:
                    p2s_bufs = [
                        psD.tile([128, 128], F32, tag=f"p2s{i}", name=f"p2s{i}")
                        for i in range(2)
                    ]
                    epi_i = [0]

                    def selfloop1(acc, b):
                        w = widths[b]
                        nc.tensor.matmul(
                            acc[b][:, :w], lhsT=hs1_t[b][:w, :],
                            rhs=ident_sb[:w, :w],
                            start=True, stop=False,
                        )

                    def epilogue1(acc, b):
                        w = widths[b]
                        t1 = hsp.tile([128, 128], F32, tag="t1", name=f"t1_{b}")
                        nc.vector.tensor_tensor(
                            out=t1[:, :w], in0=acc[b][:, :w],
                            in1=dinv_bc[:, b * BLK : b * BLK + w],
                            op=mybir.AluOpType.mult,
                        )
                        hsrT = hsp.tile([128, 128], F32, tag="hsrT",
                                        name=f"hsrT_{b}")
                        nc.scalar.activation(
                            hsrT[:, :w], t1[:, :w],
                            mybir.ActivationFunctionType.Relu,
                            bias=b1_sb[:, 0:1],
                        )
                        p2s = p2s_bufs[epi_i[0] % 2]
                        epi_i[0] += 1
                        nc.tensor.matmul(
                            p2s[:w, :OUT], lhsT=hsrT[:, :w], rhs=w2_sb[:, :],
                            start=True, stop=True,
                        )
                        t2 = constp.tile([128, 128], BF16, tag=f"hsr2_{b}",
                                         name=f"hsr2t_{b}")
                        nc.vector.memset(t2[:, OUT:], 0.0)
                        nc.scalar.activation(
                            t2[:w, :OUT], p2s[:w, :OUT],
                            mybir.ActivationFunctionType.Copy,
                            scale=dinv_sb[:w, b : b + 1],
                        )
                        nc.sync.dma_start(
                            out=hsr2_stage[b * BLK : b * BLK + w, :],
                            in_=t2[:w, :],
                        )
                        hsr2_t[b] = t2

                    run_layer(1, hs1_stage, hs1_full, HID, [128, 128], psD,
                              selfloop1, epilogue1)

                    # layer-2 collectives AFTER all layer-1 gathers: they
                    # never block the in-order gather stream; layer-2's own
                    # gathers cover their latency
                    nc.gpsimd.collective_compute(
                        "AllGather",
                        mybir.AluOpType.bypass,
                        replica_groups=rg,
                        ins=[hsr2_stage[0 : NPg[0], :].opt()],
                        outs=[hsr2_full[0][0 : ncores * NPg[0], :].opt()],
                    )
                    nc.gpsimd.collective_compute(
                        "AllGather",
                        mybir.AluOpType.bypass,
                        replica_groups=rg,
                        ins=[hsr2_stage[NPg[0] : NP, :].opt()],
                        outs=[hsr2_full[1][0 : ncores * NPg[1], :].opt()],
                    )

                # ---- layer 2: S2 -> y ----
                with tc.tile_pool(name="psF", bufs=1, space="PSUM") as psF:

                    def selfloop2(acc, b):
                        w = widths[b]
                        nc.tensor.matmul(
                            acc[b][:w, :OUT], lhsT=ident_sb[:w, :w],
                            rhs=hsr2_t[b][:w, :OUT],
                            start=True, stop=False,
                        )

                    def epilogue2(acc, b):
                        w = widths[b]
                        o1 = hsp.tile([128, OUT], F32, tag="o1", name=f"o1_{b}")
                        nc.scalar.activation(
                            o1[:w, :], acc[b][:w, :OUT],
                            mybir.ActivationFunctionType.Copy,
                            scale=dinv_sb[:w, b : b + 1],
                        )
                        yt = hsp.tile([128, OUT], F32, tag="yt", name=f"yt_{b}")
                        nc.vector.tensor_tensor(
                            out=yt[:w, :], in0=o1[:w, :], in1=b2_bc[:w, :],
                            op=mybir.AluOpType.add,
                        )
                        nc.sync.dma_start(out=y[b * BLK : b * BLK + w, :],
                                          in_=yt[:w, :])

                    run_layer(2, hsr2_stage, hsr2_full, 128, [128, 64], psF,
                              selfloop2, epilogue2)

    nc.compile()
    return nc


def _make_ident():
    import ml_dtypes

    return np.eye(128, dtype=np.float32).astype(ml_dtypes.bfloat16)


_IDENT = _make_ident()


def make_in_maps(per_core, W1, b1, W2, b2):
    import ml_dtypes

    W1 = np.ascontiguousarray(np.asarray(W1, np.float32).astype(ml_dtypes.bfloat16))
    W2 = np.ascontiguousarray(np.asarray(W2, np.float32))
    b1 = np.ascontiguousarray(np.asarray(b1, np.float32).reshape(-1, 1))
    b2 = np.asarray(b2, np.float32).reshape(1, -1)
    return [
        {
            "x_tr": pc["x_tr"],
            "w1": W1,
            "b1c": b1,
            "w2": W2,
            "b2": b2,
            "deg_own": pc["deg_own"],
            "deg_row": pc["deg_row"],
            "gidx": pc["gidx"],
            "ohs": pc["ohs"],
            "ident": _IDENT,
        }
        for pc in per_core
    ]


def kernel_run(x, edge_index, W1, b1, W2, b2, trace=False, tmpdir=None):
    x = np.ascontiguousarray(np.asarray(x, np.float32))
    per_core, meta = preprocess(x, edge_index)
    HID = np.asarray(W1).shape[1]
    OUT = np.asarray(W2).shape[1]
    nc = build_nc(meta, HID, OUT)
    in_maps = make_in_maps(per_core, W1, b1, W2, b2)
    res = run_bass_kernel_spmd(
        nc, in_maps, core_ids=list(range(NCORES)), trace=trace, tmpdir=tmpdir
    )
    out = np.concatenate([r["y"] for r in res.results], axis=0)
    return out, res


def kernel(x, edge_index, W1, b1, W2, b2):
    out, _ = kernel_run(x, edge_index, W1, b1, W2, b2)
    return out


# revision 11
# speedup vs baseline: 1.0019x; 1.0019x over previous
"""2-layer GCN encoder as a distributed Bass kernel on 8 TRN2 NeuronCores.

Decomposition (per core, nodes sharded by destination):
  hs1[v]  = dinv[v] * (x[v] @ W1)                 (own rows, bf16)
  S1T[:,d]= sum_{e: dst=d} hs1[src_e]             (dma_gather + one-hot matmul,
                                                   accumulated TRANSPOSED)
  hsrT    = relu(dinv_col * S1T + b1)             (dst-side dinv per column)
  hsr2[v] = dinv[v] * (hsrT^T @ W2)               (W2 commutes with the layer-2
                                                   edge sum -> aggregate at 64)
  S2[d]   = sum_{e: dst=d} hsr2[src_e]
  y[d]    = dinv[d]*S2[d] + b2

The critical resource is SWDGE descriptor emission (~2ns of Q7 time per
gathered row, 4 queue-pairs).  Sources are split into three streams per
destination block:
  own  - sources owned by this core, gathered from the LOCAL stage tensor;
         no collective dependency, so these gathers run inside the CC-barrier
         / AllGather windows that would otherwise idle the Q7.
  g0   - remote sources from every rank's first GSPLIT blocks (AllGather 0)
  g1   - remote sources from the remaining blocks (AllGather 1)
Each layer's AllGather runs as two chunked collectives; layer-2's triggers
are placed after ALL of layer 1's gather instructions so they never stall
the in-order GpSimd stream.  Remote segments issue with g0 lookahead
([g0 r0][g0 r1][g0 r2][g1 r0][g0 r3][g1 r1]...) so g1 consumption starts
only after its AllGather has had time to complete.

Destination blocks are processed in rounds of 6 (PSUM allocations are
bank-granular: 6 accumulators + 2 aux = 8 banks).  Gather instructions pack
up to MAXCH=16 chunks across block boundaries (single_packet=False is
required above 64 descriptors per engine).  Per-segment valid counts are
uniform across cores (num_idxs_reg); interior pads gather row 0 against
zero one-hot columns, trailing pads are -1 on every core identically.
"""

import numpy as np

import concourse.bass as bass
import concourse.bacc as bacc
import concourse.mybir as mybir
import concourse.tile as tile
from concourse import library_config
from concourse.bass_utils import run_bass_kernel_spmd

F32 = mybir.dt.float32
BF16 = mybir.dt.bfloat16
FP8 = mybir.dt.float8e4
I16 = mybir.dt.int16

NCORES = 8
BLK = 128
MAXCH = 16     # chunks (128 idx each) per dma_gather instruction
NQUEUES = 4
NS = 3         # source streams: 0,1 remote AllGather groups; 2 = own-core


def _cdiv(a, b):
    return (a + b - 1) // b


def preprocess(x, edge_index, ncores=NCORES):
    """Host-side graph partitioning: shard nodes/edges by dst, split sources
    into own/remote-g0/remote-g1 streams, build per-core gather indices and
    the one-hot chunk matrices (fp8)."""
    import ml_dtypes

    N, IN = x.shape
    assert N % ncores == 0
    NP = N // ncores
    nblk = _cdiv(NP, BLK)
    R = 6                              # blocks per processing round
    NR = _cdiv(nblk, R)
    GSPLIT = 12                        # remote source-group boundary (blocks)
    gb = [0, GSPLIT * BLK, NP]
    NPg = [gb[1] - gb[0], gb[2] - gb[1]]
    widths = [min(BLK, NP - b * BLK) for b in range(nblk)]
    rounds = [list(range(r * R, min((r + 1) * R, nblk))) for r in range(NR)]

    src = np.asarray(edge_index[0], dtype=np.int64)
    dst = np.asarray(edge_index[1], dtype=np.int64)
    deg = (np.bincount(dst, minlength=N) + 1).astype(np.float32)

    # dedupe repeated (src, dst) pairs; multiplicity goes into the multi-hot
    key = dst * N + src
    ukey, mult = np.unique(key, return_counts=True)
    dst_s = ukey // N
    src_s = ukey % N
    mult = mult.astype(np.float32)

    srcr = src_s // NP
    srco = src_s % NP
    sgrem = (srco >= gb[1]).astype(np.int64)         # remote group id
    growrem = srcr * np.asarray(NPg)[sgrem] + (srco - np.asarray(gb)[sgrem])

    bounds = np.array(
        [i * NP + b * BLK for i in range(ncores) for b in range(nblk)] + [N],
        dtype=np.int64,
    )
    pos = np.searchsorted(dst_s, bounds)

    # per (core, block, stream): sorted unique rows + scatter triplets
    blk_rows = {}
    blk_scatter = {}
    ucnt = np.zeros((ncores, nblk, NS), np.int64)
    for i in range(ncores):
        for b in range(nblk):
            k = i * nblk + b
            s0, s1 = pos[k], pos[k + 1]
            dl = (dst_s[s0:s1] - (i * NP + b * BLK)).astype(np.int64)
            own = srcr[s0:s1] == i
            stream = np.where(own, 2, sgrem[s0:s1])
            row = np.where(own, srco[s0:s1], growrem[s0:s1])
            for g in range(NS):
                m = stream == g
                urows, inv = np.unique(row[m], return_inverse=True)
                ucnt[i, b, g] = len(urows)
                blk_rows[i, b, g] = urows
                blk_scatter[i, b, g] = (inv, dl[m], mult[s0:s1][m])

    CH = np.maximum(1, _cdiv(ucnt.max(axis=0), 128)).astype(np.int64)  # [b, g]

    # chunk layout: own stream first (all blocks), then per round g0, g1
    cbase = {}
    c = 0
    for b in range(nblk):
        cbase[b, 2] = c
        c += int(CH[b, 2])
    own_end = c
    stream_span = {}
    for r in range(NR):
        for g in range(2):
            st = c
            for b in rounds[r]:
                cbase[b, g] = c
                c += int(CH[b, g])
            stream_span[r, g] = (st, c)
    NCHT = c

    # segments (stream_kind, c0, sch): own packs across all blocks; remote
    # packs within each (round, g) stream
    own_segs = [(c0, min(MAXCH, own_end - c0)) for c0 in range(0, own_end, MAXCH)]
    rem_segs = {}
    for r in range(NR):
        for g in range(2):
            st, en = stream_span[r, g]
            rem_segs[r, g] = [(c0, min(MAXCH, en - c0))
                              for c0 in range(st, en, MAXCH)]
    segments = [("own", c0, sch) for (c0, sch) in own_segs]
    for r in range(NR):
        for g in range(2):
            segments += [(g, c0, sch) for (c0, sch) in rem_segs[r, g]]

    # first pass: raw index values per core (-1 where no real source)
    all_vals = []
    all_ohs = []
    for i in range(ncores):
        vals = np.full(NCHT * 128, -1, np.int64)
        ohs = np.zeros((128, NCHT * 128), np.float32)
        for b in range(nblk):
            for g in range(NS):
                urows = blk_rows[i, b, g]
                inv, dl, mlt = blk_scatter[i, b, g]
                c0 = cbase[b, g]
                vals[c0 * 128 : c0 * 128 + len(urows)] = urows
                gc = c0 + inv // 128
                p = inv % 128
                np.add.at(ohs, (p, gc * 128 + dl), mlt)
        all_vals.append(vals)
        all_ohs.append(ohs)

    # per-segment valid count must be uniform across cores: num_idxs_reg is
    # baked into the shared program, and the ucode's trailing trim must land
    # exactly at the register value on every core
    seg_valid = []
    for (_, c0, sch) in segments:
        nv = 1
        for i in range(ncores):
            v = all_vals[i][c0 * 128 : (c0 + sch) * 128]
            nz = np.nonzero(v >= 0)[0]
            if len(nz):
                nv = max(nv, int(nz[-1]) + 1)
        seg_valid.append(nv)

    per_core = []
    for i in range(ncores):
        vals = all_vals[i]
        gidx = np.zeros((128, NCHT * 8), np.int16)
        for si, (_, c0, sch) in enumerate(segments):
            v = vals[c0 * 128 : (c0 + sch) * 128].copy()
            nv = seg_valid[si]
            v[:nv][v[:nv] < 0] = 0     # interior pads gather row 0
            v[nv:] = -1                # uniform trailing trim point
            wr = v.reshape(sch * 8, 16).T
            gidx[:, c0 * 8 : (c0 + sch) * 8] = np.tile(wr.astype(np.int16), (8, 1))

        degp = np.concatenate(
            [deg[i * NP : (i + 1) * NP], np.ones(nblk * BLK - NP, np.float32)]
        )
        per_core.append(
            {
                "x_tr": np.ascontiguousarray(
                    x[i * NP : (i + 1) * NP].T.astype(ml_dtypes.bfloat16)
                ),
                "deg_own": np.ascontiguousarray(degp.reshape(nblk, BLK).T),
                "deg_row": np.ascontiguousarray(degp.reshape(1, nblk * BLK)),
                "gidx": gidx,
                "ohs": all_ohs[i].astype(ml_dtypes.float8_e4m3),
            }
        )

    meta = {
        "N": N,
        "NP": NP,
        "IN": IN,
        "nblk": nblk,
        "R": R,
        "NR": NR,
        "GSPLIT": GSPLIT,
        "NPg": NPg,
        "rounds": rounds,
        "widths": widths,
        "CH": CH,
        "cbase": cbase,
        "own_segs": own_segs,
        "rem_segs": rem_segs,
        "segments": segments,
        "seg_valid": seg_valid,
        "NCHT": NCHT,
    }
    return per_core, meta


def build_nc(meta, HID, OUT, ncores=NCORES):
    N, NP, IN = meta["N"], meta["NP"], meta["IN"]
    nblk, widths = meta["nblk"], meta["widths"]
    R, NR, NPg, rounds = meta["R"], meta["NR"], meta["NPg"], meta["rounds"]
    GSPLIT = meta["GSPLIT"]
    CH, cbase, NCHT = meta["CH"], meta["cbase"], meta["NCHT"]
    segments, seg_valid = meta["segments"], meta["seg_valid"]
    own_segs, rem_segs = meta["own_segs"], meta["rem_segs"]
    KC = IN // 128
    assert IN % 128 == 0 and HID == 128 and OUT <= 128

    nc = bacc.Bacc(
        "TRN2",
        target_bir_lowering=False,
        debug=False,
        num_devices=ncores,
        num_swdge_queues=NQUEUES,
    )

    x_tr = nc.dram_tensor("x_tr", [IN, NP], BF16, kind="ExternalInput")
    w1 = nc.dram_tensor("w1", [IN, HID], BF16, kind="ExternalInput")
    b1c = nc.dram_tensor("b1c", [HID, 1], F32, kind="ExternalInput")
    w2 = nc.dram_tensor("w2", [HID, OUT], F32, kind="ExternalInput")
    b2 = nc.dram_tensor("b2", [1, OUT], F32, kind="ExternalInput")
    deg_own = nc.dram_tensor("deg_own", [128, nblk], F32, kind="ExternalInput")
    deg_row = nc.dram_tensor("deg_row", [1, nblk * BLK], F32, kind="ExternalInput")
    gidx_d = nc.dram_tensor("gidx", [128, NCHT * 8], I16, kind="ExternalInput")
    ident_d = nc.dram_tensor("ident", [128, 128], BF16, kind="ExternalInput")
    ohs_d = nc.dram_tensor("ohs", [128, NCHT * 128], FP8, kind="ExternalInput")
    y = nc.dram_tensor("y", [NP, OUT], F32, kind="ExternalOutput")

    hs1_stage = nc.dram_tensor("hs1_stage", [NP, HID], BF16)
    hsr2_stage = nc.dram_tensor("hsr2_stage", [NP, 128], BF16)
    hs1_full = [
        nc.dram_tensor(f"hs1_full{g}", [ncores * NPg[g], HID], BF16,
                       addr_space="Shared")
        for g in range(2)
    ]
    hsr2_full = [
        nc.dram_tensor(f"hsr2_full{g}", [ncores * NPg[g], 128], BF16,
                       addr_space="Shared")
        for g in range(2)
    ]
    rg = [list(range(ncores))]
    qn = [0]

    def next_q():
        q = qn[0]
        qn[0] = (q + 1) % NQUEUES
        return q

    # last chunk of each block (end of its g1 stream) -> stop flag
    lastc = {b: cbase[b, 1] + int(CH[b, 1]) - 1 for b in range(nblk)}
    chunk_blk = {}
    for b in range(nblk):
        for g in range(NS):
            for cc in range(int(CH[b, g])):
                chunk_blk[cbase[b, g] + cc] = b
    segv = {c0: nv for (_, c0, _), nv in zip(segments, seg_valid)}

    # remote segment issue order: g0 lookahead so g1 consumption starts
    # only after its AllGather completes
    seq = [(0, 0), (1, 0), (2, 0), (0, 1), (3, 0), (1, 1), (4, 0), (2, 1),
           (3, 1), (4, 1)]

    def remote_order():
        order = []
        for (r, g) in seq:
            order += [(g, c0, sch) for (c0, sch) in rem_segs[r, g]]
        return order

    XG = 10  # blocks per x-load slice

    with tile.TileContext(nc) as tc:
        with (
            tc.tile_pool(name="const", bufs=1) as constp,
            tc.tile_pool(name="hs", bufs=4) as hsp,
        ):
            nc.gpsimd.load_library(library_config.mlp)

            dinv_sb = constp.tile([128, nblk], F32, tag="dinv")
            b2_bc = constp.tile([128, OUT], F32, tag="b2bc")
            dinv_bc = constp.tile([128, nblk * BLK], F32, tag="dinvbc")
            gidx_sb = constp.tile([128, NCHT * 8], I16, tag="gidx")
            ident_sb = constp.tile([128, 128], BF16, tag="ident")
            w2_sb = constp.tile([HID, OUT], F32, tag="w2")
            b1_sb = constp.tile([HID, 1], F32, tag="b1")
            b2_sb = constp.tile([1, OUT], F32, tag="b2")
            ones_sb = constp.tile([1, 128], F32, tag="ones")
            ohs_sb = constp.tile([128, NCHT * 128], FP8, tag="ohs")
            hs1_t = []

            with tc.tile_pool(name="pre", bufs=1) as prep:
                # ---- loads needed by phase B ----
                xsb = {}
                for k in range(KC):
                    for s in range(0, nblk, XG):
                        cols = sum(widths[s : s + XG])
                        t = prep.tile([128, XG * BLK], BF16, tag=f"x{k}_{s}")
                        nc.sync.dma_start(
                            out=t[:, :cols],
                            in_=x_tr[k * 128 : (k + 1) * 128,
                                     s * BLK : s * BLK + cols],
                        )
                        xsb[k, s] = t
                w1c = []
                for k in range(KC):
                    t = prep.tile([128, HID], BF16, tag=f"w1c{k}")
                    nc.sync.dma_start(out=t[:], in_=w1[k * 128 : (k + 1) * 128, :])
                    w1c.append(t)
                nc.sync.dma_start(out=dinv_sb[:], in_=deg_own[:, :])
                nc.scalar.sqrt(dinv_sb[:], dinv_sb[:])
                nc.vector.reciprocal(dinv_sb[:], dinv_sb[:])

                # all remaining constants up-front on the sync HWDGE queue so
                # they never overlap (and starve) the first AllGather
                nc.sync.dma_start(out=gidx_sb[:], in_=gidx_d[:, :])
                nc.sync.dma_start(out=ident_sb[:], in_=ident_d[:, :])
                nc.sync.dma_start(out=w2_sb[:], in_=w2[:, :])
                nc.sync.dma_start(out=b1_sb[:], in_=b1c[:, :])
                nc.sync.dma_start(out=b2_sb[:], in_=b2[:, :])
                nc.vector.memset(ones_sb[:], 1.0)
                qcols = _cdiv(NCHT * 128, 4)
                for qq in range(4):
                    c0q = qq * qcols
                    c1q = min((qq + 1) * qcols, NCHT * 128)
                    nc.sync.dma_start(out=ohs_sb[:, c0q:c1q],
                                      in_=ohs_d[:, c0q:c1q])
                deg_rsb = prep.tile([1, nblk * BLK], F32, tag="degrow")
                nc.sync.dma_start(out=deg_rsb[:], in_=deg_row[:, :])

                # ---- phase B + broadcasts (own PSUM scope) ----
                with tc.tile_pool(name="psB", bufs=2, space="PSUM") as psB:
                    for b in range(nblk):
                        w = widths[b]
                        ph = psB.tile([128, HID], F32, tag="acc")
                        for k in range(KC):
                            nc.tensor.matmul(
                                ph[:w, :],
                                lhsT=xsb[k, (b // XG) * XG][
                                    :, (b % XG) * BLK : (b % XG) * BLK + w],
                                rhs=w1c[k][:, :],
                                start=(k == 0),
                                stop=(k == KC - 1),
                            )
                        t = constp.tile([128, HID], BF16, tag=f"hs1_{b}",
                                        name=f"hs1t_{b}")
                        nc.scalar.activation(
                            t[:w, :],
                            ph[:w, :],
                            mybir.ActivationFunctionType.Copy,
                            scale=dinv_sb[:w, b : b + 1],
                        )
                        nc.scalar.dma_start(
                            out=hs1_stage[b * BLK : b * BLK + w, :], in_=t[:w, :]
                        )
                        hs1_t.append(t)
                        if b == GSPLIT - 1:
                            nc.gpsimd.collective_compute(
                                "AllGather",
                                mybir.AluOpType.bypass,
                                replica_groups=rg,
                                ins=[hs1_stage[0 : NPg[0], :].opt()],
                                outs=[hs1_full[0][0 : ncores * NPg[0], :].opt()],
                            )
                    nc.gpsimd.collective_compute(
                        "AllGather",
                        mybir.AluOpType.bypass,
                        replica_groups=rg,
                        ins=[hs1_stage[NPg[0] : NP, :].opt()],
                        outs=[hs1_full[1][0 : ncores * NPg[1], :].opt()],
                    )

                    # broadcast b2 to all partitions via rank-1 matmul
                    pb2 = psB.tile([128, 128], F32, tag="aux")
                    nc.tensor.matmul(pb2[:, :OUT], lhsT=ones_sb[:], rhs=b2_sb[:],
                                     start=True, stop=True)
                    nc.vector.tensor_copy(b2_bc[:], pb2[:, :OUT])

                    # per-column dinv for the transposed layer-1 epilogue
                    for b in range(nblk):
                        pdv = psB.tile([128, 128], F32, tag="aux")
                        nc.tensor.matmul(
                            pdv[:], lhsT=ones_sb[:],
                            rhs=deg_rsb[:, b * BLK : (b + 1) * BLK],
                            start=True, stop=True,
                        )
                        nc.vector.tensor_copy(
                            dinv_bc[:, b * BLK : (b + 1) * BLK], pdv[:])
                    nc.scalar.sqrt(dinv_bc[:], dinv_bc[:])
                    nc.vector.reciprocal(dinv_bc[:], dinv_bc[:])

            with (
                tc.tile_pool(name="gath", bufs=12) as gathp,
                tc.tile_pool(name="gown", bufs=6) as gownp,
            ):
                # zero both gather pools once: trailing-trimmed lanes expose
                # stale SBUF; first use must not contain NaN-decoding garbage
                for zi in range(12):
                    zt = gathp.tile([128, MAXCH, HID], BF16, tag="g",
                                    name=f"z{zi}")
                    nc.vector.memset(zt[:, :, :], 0.0)
                for zi in range(6):
                    zt = gownp.tile([128, MAXCH, HID], BF16, tag="go",
                                    name=f"zo{zi}")
                    nc.vector.memset(zt[:, :, :], 0.0)

                def issue_gather(pool, tag, table, c0, sch, elem):
                    t = pool.tile([128, MAXCH, HID], BF16, tag=tag,
                                  name=f"gt{c0}")
                    nc.gpsimd.dma_gather(
                        t[:, :sch, :],
                        table.ap(),
                        gidx_sb[:, c0 * 8 : (c0 + sch) * 8],
                        sch * 128,
                        segv[c0],
                        elem,
                        queue_num=next_q(),
                        single_packet=False,
                    )
                    return t

                def one_mm(layer, acc, b, t, cc, gc, stop):
                    w = widths[b]
                    if layer == 1:
                        nc.tensor.matmul(
                            acc[b][:, :w],
                            lhsT=t[:, cc, :],
                            rhs=ohs_sb[:, gc * 128 : gc * 128 + w],
                            start=False,
                            stop=stop,
                        )
                    else:
                        nc.tensor.matmul(
                            acc[b][:w, :OUT],
                            lhsT=ohs_sb[:, gc * 128 : gc * 128 + w],
                            rhs=t[:, cc, :OUT],
                            start=False,
                            stop=stop,
                        )

                def run_layer(layer, stage_t, full_t, elem, acc_shape,
                              acc_pool, selfloop, epilogue):
                    # own-stream gathers first: no collective dependency
                    own_tiles = [
                        (issue_gather(gownp, "go", stage_t, c0, sch, elem),
                         c0, sch)
                        for (c0, sch) in own_segs
                    ]
                    rorder = remote_order()
                    rem_tiles = {}
                    ri = [0]

                    def consume_upto(n):
                        while ri[0] < min(n, len(rorder)):
                            g, c0, sch = rorder[ri[0]]
                            rem_tiles[c0] = issue_gather(
                                gathp, "g", full_t[g], c0, sch, elem)
                            ri[0] += 1

                    # by round r's MMs, segments through round r+1's g0 are
                    # issued (position of (r,g1) + following g0 in seq)
                    marks = []
                    for r in range(NR):
                        pos = seq.index((r, 1)) + 1
                        if pos < len(seq) and seq[pos][1] == 0:
                            pos += 1
                        marks.append(sum(len(rem_segs[rr, gg])
                                         for (rr, gg) in seq[:pos]))

                    for r in range(NR):
                        consume_upto(marks[r])
                        acc_cur = {}
                        for b in rounds[r]:
                            acc_cur[b] = acc_pool.tile(
                                acc_shape, F32, tag=f"acc{b - r * R}",
                                name=f"acc{layer}_{b}",
                            )
                            selfloop(acc_cur, b)
                        lo, hi = rounds[r][0], rounds[r][-1]
                        for (t, c0, sch) in own_tiles:
                            for cc in range(sch):
                                gc = c0 + cc
                                b = chunk_blk[gc]
                                if lo <= b <= hi:
                                    one_mm(layer, acc_cur, b, t, cc, gc, False)
                        for g in range(2):
                            for (c0, sch) in rem_segs[r, g]:
                                t = rem_tiles[c0]
                                for cc in range(sch):
                                    gc = c0 + cc
                                    b = chunk_blk[gc]
                                    one_mm(layer, acc_cur, b, t, cc, gc,
                                           gc == lastc[b])
                        for b in rounds[r]:
                            epilogue(acc_cur, b)
                    consume_upto(len(rorder))

                # ---- layer 1: S1^T -> hsr^T -> hsr2 ----
                hsr2_t = [None] * nblk
                with tc.tile_pool(name="psD", bufs=1, space="PSUM") as psD:
                    p2s_bufs = [
                        psD.tile([128, 128], F32, tag=f"p2s{i}", name=f"p2s{i}")
                        for i in range(2)
                    ]
                    epi_i = [0]

                    def selfloop1(acc, b):
                        w = widths[b]
                        nc.tensor.matmul(
                            acc[b][:, :w], lhsT=hs1_t[b][:w, :],
                            rhs=ident_sb[:w, :w],
                            start=True, stop=False,
                        )

                    def epilogue1(acc, b):
                        w = widths[b]
                        t1 = hsp.tile([128, 128], F32, tag="t1", name=f"t1_{b}")
                        nc.vector.tensor_tensor(
                            out=t1[:, :w], in0=acc[b][:, :w],
                            in1=dinv_bc[:, b * BLK : b * BLK + w],
                            op=mybir.AluOpType.mult,
                        )
                        hsrT = hsp.tile([128, 128], F32, tag="hsrT",
                                        name=f"hsrT_{b}")
                        nc.scalar.activation(
                            hsrT[:, :w], t1[:, :w],
                            mybir.ActivationFunctionType.Relu,
                            bias=b1_sb[:, 0:1],
                        )
                        p2s = p2s_bufs[epi_i[0] % 2]
                        epi_i[0] += 1
                        nc.tensor.matmul(
                            p2s[:w, :OUT], lhsT=hsrT[:, :w], rhs=w2_sb[:, :],
                            start=True, stop=True,
                        )
                        t2 = constp.tile([128, 128], BF16, tag=f"hsr2_{b}",
                                         name=f"hsr2t_{b}")
                        nc.vector.memset(t2[:, OUT:], 0.0)
                        nc.scalar.activation(
                            t2[:w, :OUT], p2s[:w, :OUT],
                            mybir.ActivationFunctionType.Copy,
                            scale=dinv_sb[:w, b : b + 1],
                        )
                        nc.scalar.dma_start(
                            out=hsr2_stage[b * BLK : b * BLK + w, :],
                            in_=t2[:w, :],
                        )
                        hsr2_t[b] = t2

                    run_layer(1, hs1_stage, hs1_full, HID, [128, 128], psD,
                              selfloop1, epilogue1)

                    # layer-2 collectives AFTER all layer-1 gathers: they
                    # never block the in-order gather stream; layer-2's own
                    # gathers cover their latency
                    nc.gpsimd.collective_compute(
                        "AllGather",
                        mybir.AluOpType.bypass,
                        replica_groups=rg,
                        ins=[hsr2_stage[0 : NPg[0], :].opt()],
                        outs=[hsr2_full[0][0 : ncores * NPg[0], :].opt()],
                    )
                    nc.gpsimd.collective_compute(
                        "AllGather",
                        mybir.AluOpType.bypass,
                        replica_groups=rg,
                        ins=[hsr2_stage[NPg[0] : NP, :].opt()],
                        outs=[hsr2_full[1][0 : ncores * NPg[1], :].opt()],
                    )

                # ---- layer 2: S2 -> y ----
                with tc.tile_pool(name="psF", bufs=1, space="PSUM") as psF:

                    def selfloop2(acc, b):
                        w = widths[b]
                        nc.tensor.matmul(
                            acc[b][:w, :OUT], lhsT=ident_sb[:w, :w],
                            rhs=hsr2_t[b][:w, :OUT],
                            start=True, stop=False,
                        )

                    def epilogue2(acc, b):
                        w = widths[b]
                        o1 = hsp.tile([128, OUT], F32, tag="o1", name=f"o1_{b}")
                        nc.scalar.activation(
                            o1[:w, :], acc[b][:w, :OUT],
                            mybir.ActivationFunctionType.Copy,
                            scale=dinv_sb[:w, b : b + 1],
                        )
                        yt = hsp.tile([128, OUT], F32, tag="yt", name=f"yt_{b}")
                        nc.vector.tensor_tensor(
                            out=yt[:w, :], in0=o1[:w, :], in1=b2_bc[:w, :],
                            op=mybir.AluOpType.add,
                        )
                        nc.scalar.dma_start(out=y[b * BLK : b * BLK + w, :],
                                              in_=yt[:w, :])

                    run_layer(2, hsr2_stage, hsr2_full, 128, [128, 64], psF,
                              selfloop2, epilogue2)

    nc.compile()
    return nc


def _make_ident():
    import ml_dtypes

    return np.eye(128, dtype=np.float32).astype(ml_dtypes.bfloat16)


_IDENT = _make_ident()


def make_in_maps(per_core, W1, b1, W2, b2):
    import ml_dtypes

    W1 = np.ascontiguousarray(np.asarray(W1, np.float32).astype(ml_dtypes.bfloat16))
    W2 = np.ascontiguousarray(np.asarray(W2, np.float32))
    b1 = np.ascontiguousarray(np.asarray(b1, np.float32).reshape(-1, 1))
    b2 = np.asarray(b2, np.float32).reshape(1, -1)
    return [
        {
            "x_tr": pc["x_tr"],
            "w1": W1,
            "b1c": b1,
            "w2": W2,
            "b2": b2,
            "deg_own": pc["deg_own"],
            "deg_row": pc["deg_row"],
            "gidx": pc["gidx"],
            "ohs": pc["ohs"],
            "ident": _IDENT,
        }
        for pc in per_core
    ]


def kernel_run(x, edge_index, W1, b1, W2, b2, trace=False, tmpdir=None):
    x = np.ascontiguousarray(np.asarray(x, np.float32))
    per_core, meta = preprocess(x, edge_index)
    HID = np.asarray(W1).shape[1]
    OUT = np.asarray(W2).shape[1]
    nc = build_nc(meta, HID, OUT)
    in_maps = make_in_maps(per_core, W1, b1, W2, b2)
    res = run_bass_kernel_spmd(
        nc, in_maps, core_ids=list(range(NCORES)), trace=trace, tmpdir=tmpdir
    )
    out = np.concatenate([r["y"] for r in res.results], axis=0)
    return out, res


def kernel(x, edge_index, W1, b1, W2, b2):
    out, _ = kernel_run(x, edge_index, W1, b1, W2, b2)
    return out


# revision 13
# speedup vs baseline: 1.0866x; 1.0846x over previous
"""2-layer GCN encoder as a distributed Bass kernel on 8 TRN2 NeuronCores.

Decomposition (per core, nodes sharded by destination):
  hs1[v]  = dinv[v] * (x[v] @ W1)                 (own rows, bf16)
  S1T[:,d]= sum_{e: dst=d} hs1[src_e]             (dma_gather + one-hot matmul,
                                                   accumulated TRANSPOSED)
  hsrT    = relu(dinv_col * S1T + b1)             (dst-side dinv per column)
  hsr2[v] = dinv[v] * (hsrT^T @ W2)               (W2 commutes with the layer-2
                                                   edge sum -> aggregate at 64)
  S2[d]   = sum_{e: dst=d} hsr2[src_e]
  y[d]    = dinv[d]*S2[d] + b2

The critical resource is SWDGE descriptor emission (~2ns of Q7 time per
gathered row, 4 queue-pairs).  Sources are split into three streams per
destination block:
  own  - sources owned by this core, gathered from the LOCAL stage tensor;
         no collective dependency, so these gathers run inside the CC-barrier
         / AllGather windows that would otherwise idle the Q7.
  g0   - remote sources from every rank's first GSPLIT blocks (AllGather 0)
  g1   - remote sources from the remaining blocks (AllGather 1)
Each layer's AllGather runs as two chunked collectives; layer-2's triggers
are placed after ALL of layer 1's gather instructions so they never stall
the in-order GpSimd stream.  Remote segments issue with g0 lookahead
([g0 r0][g0 r1][g0 r2][g1 r0][g0 r3][g1 r1]...) so g1 consumption starts
only after its AllGather has had time to complete.

Destination blocks are processed in rounds of 6 (PSUM allocations are
bank-granular: 6 accumulators + 2 aux = 8 banks).  Gather instructions pack
up to MAXCH=16 chunks across block boundaries (single_packet=False is
required above 64 descriptors per engine).  Per-segment valid counts are
uniform across cores (num_idxs_reg); interior pads gather row 0 against
zero one-hot columns, trailing pads are -1 on every core identically.
"""

import numpy as np

import concourse.bass as bass
import concourse.bacc as bacc
import concourse.mybir as mybir
import concourse.tile as tile
from concourse import library_config
from concourse.bass_utils import run_bass_kernel_spmd

F32 = mybir.dt.float32
BF16 = mybir.dt.bfloat16
FP8 = mybir.dt.float8e4
I16 = mybir.dt.int16

NCORES = 8
BLK = 128
MAXCH = 16     # chunks (128 idx each) per dma_gather instruction
NQUEUES = 4
NS = 2         # source streams: 0 = remote (AllGather table); 1 = own-core


def _cdiv(a, b):
    return (a + b - 1) // b


def preprocess(x, edge_index, ncores=NCORES):
    """Host-side graph partitioning: shard nodes/edges by dst, split sources
    into own/remote-g0/remote-g1 streams, build per-core gather indices and
    the one-hot chunk matrices (fp8)."""
    import ml_dtypes

    N, IN = x.shape
    assert N % ncores == 0
    NP = N // ncores
    nblk = _cdiv(NP, BLK)
    R = 6                              # blocks per processing round
    NR = _cdiv(nblk, R)
    widths = [min(BLK, NP - b * BLK) for b in range(nblk)]
    rounds = [list(range(r * R, min((r + 1) * R, nblk))) for r in range(NR)]

    src = np.asarray(edge_index[0], dtype=np.int64)
    dst = np.asarray(edge_index[1], dtype=np.int64)
    deg = (np.bincount(dst, minlength=N) + 1).astype(np.float32)

    # dedupe repeated (src, dst) pairs; multiplicity goes into the multi-hot
    key = dst * N + src
    ukey, mult = np.unique(key, return_counts=True)
    dst_s = ukey // N
    src_s = ukey % N
    mult = mult.astype(np.float32)

    srcr = src_s // NP
    srco = src_s % NP

    bounds = np.array(
        [i * NP + b * BLK for i in range(ncores) for b in range(nblk)] + [N],
        dtype=np.int64,
    )
    pos = np.searchsorted(dst_s, bounds)

    # per (core, block, stream): sorted unique rows + scatter triplets
    blk_rows = {}
    blk_scatter = {}
    ucnt = np.zeros((ncores, nblk, NS), np.int64)
    for i in range(ncores):
        for b in range(nblk):
            k = i * nblk + b
            s0, s1 = pos[k], pos[k + 1]
            dl = (dst_s[s0:s1] - (i * NP + b * BLK)).astype(np.int64)
            own = srcr[s0:s1] == i
            stream = np.where(own, 1, 0)
            # remote rows index the full AllGather table (global node id);
            # own rows index the local stage tensor
            row = np.where(own, srco[s0:s1], src_s[s0:s1])
            for g in range(NS):
                m = stream == g
                urows, inv = np.unique(row[m], return_inverse=True)
                ucnt[i, b, g] = len(urows)
                blk_rows[i, b, g] = urows
                blk_scatter[i, b, g] = (inv, dl[m], mult[s0:s1][m])

    CH = np.maximum(1, _cdiv(ucnt.max(axis=0), 128)).astype(np.int64)  # [b, g]

    # chunk layout: own stream first (all blocks), then remote per round
    cbase = {}
    c = 0
    for b in range(nblk):
        cbase[b, 1] = c
        c += int(CH[b, 1])
    own_end = c
    stream_span = {}
    for r in range(NR):
        st = c
        for b in rounds[r]:
            cbase[b, 0] = c
            c += int(CH[b, 0])
        stream_span[r] = (st, c)
    NCHT = c

    # segments (stream_kind, c0, sch): own packs across all blocks; remote
    # packs within each round's stream
    own_segs = [(c0, min(MAXCH, own_end - c0)) for c0 in range(0, own_end, MAXCH)]
    rem_segs = {}
    for r in range(NR):
        st, en = stream_span[r]
        rem_segs[r] = [(c0, min(MAXCH, en - c0))
                       for c0 in range(st, en, MAXCH)]
    segments = [("own", c0, sch) for (c0, sch) in own_segs]
    for r in range(NR):
        segments += [(0, c0, sch) for (c0, sch) in rem_segs[r]]

    # first pass: raw index values per core (-1 where no real source)
    all_vals = []
    all_ohs = []
    for i in range(ncores):
        vals = np.full(NCHT * 128, -1, np.int64)
        ohs = np.zeros((128, NCHT * 128), np.float32)
        for b in range(nblk):
            for g in range(NS):
                urows = blk_rows[i, b, g]
                inv, dl, mlt = blk_scatter[i, b, g]
                c0 = cbase[b, g]
                vals[c0 * 128 : c0 * 128 + len(urows)] = urows
                gc = c0 + inv // 128
                p = inv % 128
                np.add.at(ohs, (p, gc * 128 + dl), mlt)
        all_vals.append(vals)
        all_ohs.append(ohs)

    # per-segment valid count must be uniform across cores: num_idxs_reg is
    # baked into the shared program, and the ucode's trailing trim must land
    # exactly at the register value on every core
    seg_valid = []
    for (_, c0, sch) in segments:
        nv = 1
        for i in range(ncores):
            v = all_vals[i][c0 * 128 : (c0 + sch) * 128]
            nz = np.nonzero(v >= 0)[0]
            if len(nz):
                nv = max(nv, int(nz[-1]) + 1)
        seg_valid.append(nv)

    per_core = []
    for i in range(ncores):
        vals = all_vals[i]
        gidx = np.zeros((128, NCHT * 8), np.int16)
        for si, (_, c0, sch) in enumerate(segments):
            v = vals[c0 * 128 : (c0 + sch) * 128].copy()
            nv = seg_valid[si]
            v[:nv][v[:nv] < 0] = 0     # interior pads gather row 0
            v[nv:] = -1                # uniform trailing trim point
            wr = v.reshape(sch * 8, 16).T
            gidx[:, c0 * 8 : (c0 + sch) * 8] = np.tile(wr.astype(np.int16), (8, 1))

        degp = np.concatenate(
            [deg[i * NP : (i + 1) * NP], np.ones(nblk * BLK - NP, np.float32)]
        )
        per_core.append(
            {
                "x_tr": np.ascontiguousarray(
                    x[i * NP : (i + 1) * NP].T.astype(ml_dtypes.bfloat16)
                ),
                "deg_own": np.ascontiguousarray(degp.reshape(nblk, BLK).T),
                "deg_row": np.ascontiguousarray(degp.reshape(1, nblk * BLK)),
                "gidx": gidx,
                "ohs": all_ohs[i].astype(ml_dtypes.float8_e4m3),
            }
        )

    meta = {
        "N": N,
        "NP": NP,
        "IN": IN,
        "nblk": nblk,
        "R": R,
        "NR": NR,
        "rounds": rounds,
        "widths": widths,
        "CH": CH,
        "cbase": cbase,
        "own_segs": own_segs,
        "rem_segs": rem_segs,
        "segments": segments,
        "seg_valid": seg_valid,
        "NCHT": NCHT,
    }
    return per_core, meta


def build_nc(meta, HID, OUT, ncores=NCORES):
    N, NP, IN = meta["N"], meta["NP"], meta["IN"]
    nblk, widths = meta["nblk"], meta["widths"]
    R, NR, rounds = meta["R"], meta["NR"], meta["rounds"]
    CH, cbase, NCHT = meta["CH"], meta["cbase"], meta["NCHT"]
    segments, seg_valid = meta["segments"], meta["seg_valid"]
    own_segs, rem_segs = meta["own_segs"], meta["rem_segs"]
    KC = IN // 128
    assert IN % 128 == 0 and HID == 128 and OUT <= 128

    nc = bacc.Bacc(
        "TRN2",
        target_bir_lowering=False,
        debug=False,
        num_devices=ncores,
        num_swdge_queues=NQUEUES,
    )

    x_tr = nc.dram_tensor("x_tr", [IN, NP], BF16, kind="ExternalInput")
    w1 = nc.dram_tensor("w1", [IN, HID], BF16, kind="ExternalInput")
    b1c = nc.dram_tensor("b1c", [HID, 1], F32, kind="ExternalInput")
    w2 = nc.dram_tensor("w2", [HID, OUT], F32, kind="ExternalInput")
    b2 = nc.dram_tensor("b2", [1, OUT], F32, kind="ExternalInput")
    deg_own = nc.dram_tensor("deg_own", [128, nblk], F32, kind="ExternalInput")
    deg_row = nc.dram_tensor("deg_row", [1, nblk * BLK], F32, kind="ExternalInput")
    gidx_d = nc.dram_tensor("gidx", [128, NCHT * 8], I16, kind="ExternalInput")
    ident_d = nc.dram_tensor("ident", [128, 128], BF16, kind="ExternalInput")
    ohs_d = nc.dram_tensor("ohs", [128, NCHT * 128], FP8, kind="ExternalInput")
    y = nc.dram_tensor("y", [NP, OUT], F32, kind="ExternalOutput")

    hs1_stage = nc.dram_tensor("hs1_stage", [NP, HID], BF16)
    hsr2_stage = nc.dram_tensor("hsr2_stage", [NP, 128], BF16)
    hs1_full = nc.dram_tensor("hs1_full", [N, HID], BF16, addr_space="Shared")
    hsr2_full = nc.dram_tensor("hsr2_full", [N, 128], BF16, addr_space="Shared")
    rg = [list(range(ncores))]
    qn = [0]

    def next_q():
        q = qn[0]
        qn[0] = (q + 1) % NQUEUES
        return q

    # last chunk of each block (end of its remote stream) -> stop flag
    lastc = {b: cbase[b, 0] + int(CH[b, 0]) - 1 for b in range(nblk)}
    chunk_blk = {}
    for b in range(nblk):
        for g in range(NS):
            for cc in range(int(CH[b, g])):
                chunk_blk[cbase[b, g] + cc] = b
    segv = {c0: nv for (_, c0, _), nv in zip(segments, seg_valid)}

    def remote_order():
        order = []
        for r in range(NR):
            order += [(c0, sch) for (c0, sch) in rem_segs[r]]
        return order

    XG = 10  # blocks per x-load slice

    with tile.TileContext(nc) as tc:
        with (
            tc.tile_pool(name="const", bufs=1) as constp,
            tc.tile_pool(name="hs", bufs=4) as hsp,
        ):
            nc.gpsimd.load_library(library_config.mlp)

            dinv_sb = constp.tile([128, nblk], F32, tag="dinv")
            b2_bc = constp.tile([128, OUT], F32, tag="b2bc")
            dinv_bc = constp.tile([128, nblk * BLK], F32, tag="dinvbc")
            gidx_sb = constp.tile([128, NCHT * 8], I16, tag="gidx")
            ident_sb = constp.tile([128, 128], BF16, tag="ident")
            w2_sb = constp.tile([HID, OUT], F32, tag="w2")
            b1_sb = constp.tile([HID, 1], F32, tag="b1")
            b2_sb = constp.tile([1, OUT], F32, tag="b2")
            ones_sb = constp.tile([1, 128], F32, tag="ones")
            ohs_sb = constp.tile([128, NCHT * 128], FP8, tag="ohs")
            hs1_t = []

            with tc.tile_pool(name="pre", bufs=1) as prep:
                # ---- loads needed by phase B ----
                xsb = {}
                for k in range(KC):
                    for s in range(0, nblk, XG):
                        cols = sum(widths[s : s + XG])
                        t = prep.tile([128, XG * BLK], BF16, tag=f"x{k}_{s}")
                        nc.sync.dma_start(
                            out=t[:, :cols],
                            in_=x_tr[k * 128 : (k + 1) * 128,
                                     s * BLK : s * BLK + cols],
                        )
                        xsb[k, s] = t
                w1c = []
                for k in range(KC):
                    t = prep.tile([128, HID], BF16, tag=f"w1c{k}")
                    nc.sync.dma_start(out=t[:], in_=w1[k * 128 : (k + 1) * 128, :])
                    w1c.append(t)
                nc.sync.dma_start(out=dinv_sb[:], in_=deg_own[:, :])
                nc.scalar.sqrt(dinv_sb[:], dinv_sb[:])
                nc.vector.reciprocal(dinv_sb[:], dinv_sb[:])

                # all remaining constants up-front on the sync HWDGE queue so
                # they never overlap (and starve) the first AllGather
                nc.sync.dma_start(out=gidx_sb[:], in_=gidx_d[:, :])
                nc.sync.dma_start(out=ident_sb[:], in_=ident_d[:, :])
                nc.sync.dma_start(out=w2_sb[:], in_=w2[:, :])
                nc.sync.dma_start(out=b1_sb[:], in_=b1c[:, :])
                nc.sync.dma_start(out=b2_sb[:], in_=b2[:, :])
                nc.vector.memset(ones_sb[:], 1.0)
                qcols = _cdiv(NCHT * 128, 4)
                for qq in range(4):
                    c0q = qq * qcols
                    c1q = min((qq + 1) * qcols, NCHT * 128)
                    nc.sync.dma_start(out=ohs_sb[:, c0q:c1q],
                                      in_=ohs_d[:, c0q:c1q])
                deg_rsb = prep.tile([1, nblk * BLK], F32, tag="degrow")
                nc.sync.dma_start(out=deg_rsb[:], in_=deg_row[:, :])

                # ---- phase B + broadcasts (own PSUM scope) ----
                with tc.tile_pool(name="psB", bufs=2, space="PSUM") as psB:
                    for b in range(nblk):
                        w = widths[b]
                        ph = psB.tile([128, HID], F32, tag="acc")
                        for k in range(KC):
                            nc.tensor.matmul(
                                ph[:w, :],
                                lhsT=xsb[k, (b // XG) * XG][
                                    :, (b % XG) * BLK : (b % XG) * BLK + w],
                                rhs=w1c[k][:, :],
                                start=(k == 0),
                                stop=(k == KC - 1),
                            )
                        t = constp.tile([128, HID], BF16, tag=f"hs1_{b}",
                                        name=f"hs1t_{b}")
                        nc.scalar.activation(
                            t[:w, :],
                            ph[:w, :],
                            mybir.ActivationFunctionType.Copy,
                            scale=dinv_sb[:w, b : b + 1],
                        )
                        nc.scalar.dma_start(
                            out=hs1_stage[b * BLK : b * BLK + w, :], in_=t[:w, :]
                        )
                        hs1_t.append(t)

                    # broadcast b2 to all partitions via rank-1 matmul
                    pb2 = psB.tile([128, 128], F32, tag="aux")
                    nc.tensor.matmul(pb2[:, :OUT], lhsT=ones_sb[:], rhs=b2_sb[:],
                                     start=True, stop=True)
                    nc.vector.tensor_copy(b2_bc[:], pb2[:, :OUT])

                    # per-column dinv for the transposed layer-1 epilogue
                    for b in range(nblk):
                        pdv = psB.tile([128, 128], F32, tag="aux")
                        nc.tensor.matmul(
                            pdv[:], lhsT=ones_sb[:],
                            rhs=deg_rsb[:, b * BLK : (b + 1) * BLK],
                            start=True, stop=True,
                        )
                        nc.vector.tensor_copy(
                            dinv_bc[:, b * BLK : (b + 1) * BLK], pdv[:])
                    nc.scalar.sqrt(dinv_bc[:], dinv_bc[:])
                    nc.vector.reciprocal(dinv_bc[:], dinv_bc[:])

            with (
                tc.tile_pool(name="gath", bufs=12) as gathp,
                tc.tile_pool(name="gown", bufs=6) as gownp,
            ):
                # zero both gather pools once: trailing-trimmed lanes expose
                # stale SBUF; first use must not contain NaN-decoding garbage
                for zi in range(12):
                    zt = gathp.tile([128, MAXCH, HID], BF16, tag="g",
                                    name=f"z{zi}")
                    nc.vector.memset(zt[:, :, :], 0.0)
                for zi in range(6):
                    zt = gownp.tile([128, MAXCH, HID], BF16, tag="go",
                                    name=f"zo{zi}")
                    nc.vector.memset(zt[:, :, :], 0.0)

                def issue_gather(pool, tag, table, c0, sch, elem):
                    t = pool.tile([128, MAXCH, HID], BF16, tag=tag,
                                  name=f"gt{c0}")
                    nc.gpsimd.dma_gather(
                        t[:, :sch, :],
                        table.ap(),
                        gidx_sb[:, c0 * 8 : (c0 + sch) * 8],
                        sch * 128,
                        segv[c0],
                        elem,
                        queue_num=next_q(),
                        single_packet=False,
                    )
                    return t

                def one_mm(layer, acc, b, t, cc, gc, stop):
                    w = widths[b]
                    if layer == 1:
                        nc.tensor.matmul(
                            acc[b][:, :w],
                            lhsT=t[:, cc, :],
                            rhs=ohs_sb[:, gc * 128 : gc * 128 + w],
                            start=False,
                            stop=stop,
                        )
                    else:
                        nc.tensor.matmul(
                            acc[b][:w, :OUT],
                            lhsT=ohs_sb[:, gc * 128 : gc * 128 + w],
                            rhs=t[:, cc, :OUT],
                            start=False,
                            stop=stop,
                        )

                def run_layer(layer, stage_t, full_t, elem, acc_shape,
                              acc_pool, selfloop, epilogue, collective):
                    # own-stream gathers first: no collective dependency --
                    # they run inside the barrier / preceding-AG window
                    own_tiles = [
                        (issue_gather(gownp, "go", stage_t, c0, sch, elem),
                         c0, sch)
                        for (c0, sch) in own_segs
                    ]
                    # the AllGather runs exclusively (concurrent SWDGE gather
                    # traffic throttles the CC DMA ~5x)
                    collective()
                    rorder = remote_order()
                    rem_tiles = {}
                    ri = [0]

                    def consume_upto(n):
                        while ri[0] < min(n, len(rorder)):
                            c0, sch = rorder[ri[0]]
                            rem_tiles[c0] = issue_gather(
                                gathp, "g", full_t, c0, sch, elem)
                            ri[0] += 1

                    marks = []
                    done = 0
                    for r in range(NR):
                        done += len(rem_segs[r])
                        nxt = len(rem_segs[r + 1]) if r + 1 < NR else 0
                        marks.append(done + nxt)

                    for r in range(NR):
                        consume_upto(marks[r])
                        acc_cur = {}
                        for b in rounds[r]:
                            acc_cur[b] = acc_pool.tile(
                                acc_shape, F32, tag=f"acc{b - r * R}",
                                name=f"acc{layer}_{b}",
                            )
                            selfloop(acc_cur, b)
                        lo, hi = rounds[r][0], rounds[r][-1]
                        for (t, c0, sch) in own_tiles:
                            for cc in range(sch):
                                gc = c0 + cc
                                b = chunk_blk[gc]
                                if lo <= b <= hi:
                                    one_mm(layer, acc_cur, b, t, cc, gc, False)
                        for (c0, sch) in rem_segs[r]:
                            t = rem_tiles[c0]
                            for cc in range(sch):
                                gc = c0 + cc
                                b = chunk_blk[gc]
                                one_mm(layer, acc_cur, b, t, cc, gc,
                                       gc == lastc[b])
                        for b in rounds[r]:
                            epilogue(acc_cur, b)
                    consume_upto(len(rorder))

                # ---- layer 1: S1^T -> hsr^T -> hsr2 ----
                hsr2_t = [None] * nblk
                with tc.tile_pool(name="psD", bufs=1, space="PSUM") as psD:
                    p2s_bufs = [
                        psD.tile([128, 128], F32, tag=f"p2s{i}", name=f"p2s{i}")
                        for i in range(2)
                    ]
                    epi_i = [0]

                    def selfloop1(acc, b):
                        w = widths[b]
                        nc.tensor.matmul(
                            acc[b][:, :w], lhsT=hs1_t[b][:w, :],
                            rhs=ident_sb[:w, :w],
                            start=True, stop=False,
                        )

                    def epilogue1(acc, b):
                        w = widths[b]
                        t1 = hsp.tile([128, 128], F32, tag="t1", name=f"t1_{b}")
                        nc.vector.tensor_tensor(
                            out=t1[:, :w], in0=acc[b][:, :w],
                            in1=dinv_bc[:, b * BLK : b * BLK + w],
                            op=mybir.AluOpType.mult,
                        )
                        hsrT = hsp.tile([128, 128], F32, tag="hsrT",
                                        name=f"hsrT_{b}")
                        nc.scalar.activation(
                            hsrT[:, :w], t1[:, :w],
                            mybir.ActivationFunctionType.Relu,
                            bias=b1_sb[:, 0:1],
                        )
                        p2s = p2s_bufs[epi_i[0] % 2]
                        epi_i[0] += 1
                        nc.tensor.matmul(
                            p2s[:w, :OUT], lhsT=hsrT[:, :w], rhs=w2_sb[:, :],
                            start=True, stop=True,
                        )
                        t2 = constp.tile([128, 128], BF16, tag=f"hsr2_{b}",
                                         name=f"hsr2t_{b}")
                        nc.vector.memset(t2[:, OUT:], 0.0)
                        nc.scalar.activation(
                            t2[:w, :OUT], p2s[:w, :OUT],
                            mybir.ActivationFunctionType.Copy,
                            scale=dinv_sb[:w, b : b + 1],
                        )
                        nc.scalar.dma_start(
                            out=hsr2_stage[b * BLK : b * BLK + w, :],
                            in_=t2[:w, :],
                        )
                        hsr2_t[b] = t2

                    def ag1():
                        nc.gpsimd.collective_compute(
                            "AllGather",
                            mybir.AluOpType.bypass,
                            replica_groups=rg,
                            ins=[hs1_stage[0:NP, :].opt()],
                            outs=[hs1_full[0 : ncores * NP, :].opt()],
                        )

                    run_layer(1, hs1_stage, hs1_full, HID, [128, 128], psD,
                              selfloop1, epilogue1, ag1)

                # ---- layer 2: S2 -> y ----
                with tc.tile_pool(name="psF", bufs=1, space="PSUM") as psF:

                    def selfloop2(acc, b):
                        w = widths[b]
                        nc.tensor.matmul(
                            acc[b][:w, :OUT], lhsT=ident_sb[:w, :w],
                            rhs=hsr2_t[b][:w, :OUT],
                            start=True, stop=False,
                        )

                    def epilogue2(acc, b):
                        w = widths[b]
                        o1 = hsp.tile([128, OUT], F32, tag="o1", name=f"o1_{b}")
                        nc.scalar.activation(
                            o1[:w, :], acc[b][:w, :OUT],
                            mybir.ActivationFunctionType.Copy,
                            scale=dinv_sb[:w, b : b + 1],
                        )
                        yt = hsp.tile([128, OUT], F32, tag="yt", name=f"yt_{b}")
                        nc.vector.tensor_tensor(
                            out=yt[:w, :], in0=o1[:w, :], in1=b2_bc[:w, :],
                            op=mybir.AluOpType.add,
                        )
                        nc.scalar.dma_start(out=y[b * BLK : b * BLK + w, :],
                                              in_=yt[:w, :])

                    def ag2():
                        nc.gpsimd.collective_compute(
                            "AllGather",
                            mybir.AluOpType.bypass,
                            replica_groups=rg,
                            ins=[hsr2_stage[0:NP, :].opt()],
                            outs=[hsr2_full[0 : ncores * NP, :].opt()],
                        )

                    run_layer(2, hsr2_stage, hsr2_full, 128, [128, 64], psF,
                              selfloop2, epilogue2, ag2)

    nc.compile()
    return nc


def _make_ident():
    import ml_dtypes

    return np.eye(128, dtype=np.float32).astype(ml_dtypes.bfloat16)


_IDENT = _make_ident()


def make_in_maps(per_core, W1, b1, W2, b2):
    import ml_dtypes

    W1 = np.ascontiguousarray(np.asarray(W1, np.float32).astype(ml_dtypes.bfloat16))
    W2 = np.ascontiguousarray(np.asarray(W2, np.float32))
    b1 = np.ascontiguousarray(np.asarray(b1, np.float32).reshape(-1, 1))
    b2 = np.asarray(b2, np.float32).reshape(1, -1)
    return [
        {
            "x_tr": pc["x_tr"],
            "w1": W1,
            "b1c": b1,
            "w2": W2,
            "b2": b2,
            "deg_own": pc["deg_own"],
            "deg_row": pc["deg_row"],
            "gidx": pc["gidx"],
            "ohs": pc["ohs"],
            "ident": _IDENT,
        }
        for pc in per_core
    ]


def kernel_run(x, edge_index, W1, b1, W2, b2, trace=False, tmpdir=None):
    x = np.ascontiguousarray(np.asarray(x, np.float32))
    per_core, meta = preprocess(x, edge_index)
    HID = np.asarray(W1).shape[1]
    OUT = np.asarray(W2).shape[1]
    nc = build_nc(meta, HID, OUT)
    in_maps = make_in_maps(per_core, W1, b1, W2, b2)
    res = run_bass_kernel_spmd(
        nc, in_maps, core_ids=list(range(NCORES)), trace=trace, tmpdir=tmpdir
    )
    out = np.concatenate([r["y"] for r in res.results], axis=0)
    return out, res


def kernel(x, edge_index, W1, b1, W2, b2):
    out, _ = kernel_run(x, edge_index, W1, b1, W2, b2)
    return out


# revision 16
# speedup vs baseline: 1.2422x; 1.1432x over previous
"""2-layer GCN encoder as a distributed Bass kernel on 8 TRN2 NeuronCores.

Decomposition (per core, nodes sharded by destination):
  hs1[v]  = dinv[v] * (x[v] @ W1)                 (own rows, bf16)
  S1T[:,d]= sum_{e: dst=d} hs1[src_e]             (dma_gather + one-hot matmul,
                                                   accumulated TRANSPOSED)
  hsrT    = relu(dinv_col * S1T + b1)             (dst-side dinv per column)
  hsr2[v] = dinv[v] * (hsrT^T @ W2)               (W2 commutes with the layer-2
                                                   edge sum -> aggregate at 64)
  S2[d]   = sum_{e: dst=d} hsr2[src_e]
  y[d]    = dinv[d]*S2[d] + b2

The critical resource is SWDGE descriptor emission (~2ns of Q7 time per
gathered row, 4 queue-pairs).  Sources are split into three streams per
destination block:
  own  - sources owned by this core, gathered from the LOCAL stage tensor;
         no collective dependency, so these gathers run inside the CC-barrier
         / AllGather windows that would otherwise idle the Q7.
  g0   - remote sources from every rank's first GSPLIT blocks (AllGather 0)
  g1   - remote sources from the remaining blocks (AllGather 1)
Each layer's AllGather runs as two chunked collectives; layer-2's triggers
are placed after ALL of layer 1's gather instructions so they never stall
the in-order GpSimd stream.  Remote segments issue with g0 lookahead
([g0 r0][g0 r1][g0 r2][g1 r0][g0 r3][g1 r1]...) so g1 consumption starts
only after its AllGather has had time to complete.

Destination blocks are processed in rounds of 6 (PSUM allocations are
bank-granular: 6 accumulators + 2 aux = 8 banks).  Gather instructions pack
up to MAXCH=16 chunks across block boundaries (single_packet=False is
required above 64 descriptors per engine).  Per-segment valid counts are
uniform across cores (num_idxs_reg); interior pads gather row 0 against
zero one-hot columns, trailing pads are -1 on every core identically.
"""

import numpy as np

import concourse.bass as bass
import concourse.bacc as bacc
import concourse.mybir as mybir
import concourse.tile as tile
from concourse import library_config
from concourse.bass_utils import run_bass_kernel_spmd

F32 = mybir.dt.float32
BF16 = mybir.dt.bfloat16
FP8 = mybir.dt.float8e4
I16 = mybir.dt.int16

NCORES = 8
BLK = 128
MAXCH = 16     # chunks (128 idx each) per dma_gather instruction
NQUEUES = 4
NS = 1         # single source stream per block (full AllGather table)


def _cdiv(a, b):
    return (a + b - 1) // b


def preprocess(x, edge_index, ncores=NCORES):
    """Host-side graph partitioning: shard nodes/edges by dst, split sources
    into own/remote-g0/remote-g1 streams, build per-core gather indices and
    the one-hot chunk matrices (fp8)."""
    import ml_dtypes

    N, IN = x.shape
    assert N % ncores == 0
    NP = N // ncores
    nblk = _cdiv(NP, BLK)
    R = 6                              # blocks per processing round
    NR = _cdiv(nblk, R)
    widths = [min(BLK, NP - b * BLK) for b in range(nblk)]
    rounds = [list(range(r * R, min((r + 1) * R, nblk))) for r in range(NR)]

    src = np.asarray(edge_index[0], dtype=np.int64)
    dst = np.asarray(edge_index[1], dtype=np.int64)
    deg = (np.bincount(dst, minlength=N) + 1).astype(np.float32)

    # dedupe repeated (src, dst) pairs; multiplicity goes into the multi-hot
    key = dst * N + src
    ukey, mult = np.unique(key, return_counts=True)
    dst_s = ukey // N
    src_s = ukey % N
    mult = mult.astype(np.float32)

    srcr = src_s // NP
    srco = src_s % NP

    bounds = np.array(
        [i * NP + b * BLK for i in range(ncores) for b in range(nblk)] + [N],
        dtype=np.int64,
    )
    pos = np.searchsorted(dst_s, bounds)

    # per (core, block, stream): sorted unique rows + scatter triplets
    blk_rows = {}
    blk_scatter = {}
    ucnt = np.zeros((ncores, nblk, NS), np.int64)
    for i in range(ncores):
        for b in range(nblk):
            k = i * nblk + b
            s0, s1 = pos[k], pos[k + 1]
            dl = (dst_s[s0:s1] - (i * NP + b * BLK)).astype(np.int64)
            urows, inv = np.unique(src_s[s0:s1], return_inverse=True)
            ucnt[i, b, 0] = len(urows)
            blk_rows[i, b, 0] = urows
            blk_scatter[i, b, 0] = (inv, dl, mult[s0:s1])

    CH = np.maximum(1, _cdiv(ucnt.max(axis=0), 128)).astype(np.int64)  # [b, g]

    # chunk layout: per round, blocks in order
    cbase = {}
    c = 0
    stream_span = {}
    for r in range(NR):
        st = c
        for b in rounds[r]:
            cbase[b, 0] = c
            c += int(CH[b, 0])
        stream_span[r] = (st, c)
    NCHT = c

    # segments pack MAXCH chunks within each round's stream
    rem_segs = {}
    for r in range(NR):
        st, en = stream_span[r]
        rem_segs[r] = [(c0, min(MAXCH, en - c0))
                       for c0 in range(st, en, MAXCH)]
    segments = []
    for r in range(NR):
        segments += [(0, c0, sch) for (c0, sch) in rem_segs[r]]

    # first pass: raw index values per core (-1 where no real source)
    all_vals = []
    all_ohs = []
    for i in range(ncores):
        vals = np.full(NCHT * 128, -1, np.int64)
        ohs = np.zeros((128, NCHT * 128), np.float32)
        for b in range(nblk):
            for g in range(NS):
                urows = blk_rows[i, b, g]
                inv, dl, mlt = blk_scatter[i, b, g]
                c0 = cbase[b, g]
                vals[c0 * 128 : c0 * 128 + len(urows)] = urows
                gc = c0 + inv // 128
                p = inv % 128
                np.add.at(ohs, (p, gc * 128 + dl), mlt)
        all_vals.append(vals)
        all_ohs.append(ohs)

    # per-segment valid count must be uniform across cores: num_idxs_reg is
    # baked into the shared program, and the ucode's trailing trim must land
    # exactly at the register value on every core
    seg_valid = []
    for (_, c0, sch) in segments:
        nv = 1
        for i in range(ncores):
            v = all_vals[i][c0 * 128 : (c0 + sch) * 128]
            nz = np.nonzero(v >= 0)[0]
            if len(nz):
                nv = max(nv, int(nz[-1]) + 1)
        seg_valid.append(nv)

    per_core = []
    for i in range(ncores):
        vals = all_vals[i]
        gidx = np.zeros((128, NCHT * 8), np.int16)
        for si, (_, c0, sch) in enumerate(segments):
            v = vals[c0 * 128 : (c0 + sch) * 128].copy()
            nv = seg_valid[si]
            v[:nv][v[:nv] < 0] = 0     # interior pads gather row 0
            v[nv:] = -1                # uniform trailing trim point
            wr = v.reshape(sch * 8, 16).T
            gidx[:, c0 * 8 : (c0 + sch) * 8] = np.tile(wr.astype(np.int16), (8, 1))

        degp = np.concatenate(
            [deg[i * NP : (i + 1) * NP], np.ones(nblk * BLK - NP, np.float32)]
        )
        per_core.append(
            {
                "x_tr": np.ascontiguousarray(
                    x[i * NP : (i + 1) * NP].T.astype(ml_dtypes.bfloat16)
                ),
                "deg_own": np.ascontiguousarray(degp.reshape(nblk, BLK).T),
                "deg_row": np.ascontiguousarray(degp.reshape(1, nblk * BLK)),
                "gidx": gidx,
                "ohs": all_ohs[i].astype(ml_dtypes.float8_e4m3),
            }
        )

    meta = {
        "N": N,
        "NP": NP,
        "IN": IN,
        "nblk": nblk,
        "R": R,
        "NR": NR,
        "rounds": rounds,
        "widths": widths,
        "CH": CH,
        "cbase": cbase,
        "rem_segs": rem_segs,
        "segments": segments,
        "seg_valid": seg_valid,
        "NCHT": NCHT,
    }
    return per_core, meta


def build_nc(meta, HID, OUT, ncores=NCORES):
    N, NP, IN = meta["N"], meta["NP"], meta["IN"]
    nblk, widths = meta["nblk"], meta["widths"]
    R, NR, rounds = meta["R"], meta["NR"], meta["rounds"]
    CH, cbase, NCHT = meta["CH"], meta["cbase"], meta["NCHT"]
    segments, seg_valid = meta["segments"], meta["seg_valid"]
    rem_segs = meta["rem_segs"]
    KC = IN // 128
    assert IN % 128 == 0 and HID == 128 and OUT <= 128

    nc = bacc.Bacc(
        "TRN2",
        target_bir_lowering=False,
        debug=False,
        num_devices=ncores,
        num_swdge_queues=NQUEUES,
    )

    x_tr = nc.dram_tensor("x_tr", [IN, NP], BF16, kind="ExternalInput")
    w1 = nc.dram_tensor("w1", [IN, HID], BF16, kind="ExternalInput")
    b1c = nc.dram_tensor("b1c", [HID, 1], F32, kind="ExternalInput")
    w2 = nc.dram_tensor("w2", [HID, OUT], F32, kind="ExternalInput")
    b2 = nc.dram_tensor("b2", [1, OUT], F32, kind="ExternalInput")
    deg_own = nc.dram_tensor("deg_own", [128, nblk], F32, kind="ExternalInput")
    deg_row = nc.dram_tensor("deg_row", [1, nblk * BLK], F32, kind="ExternalInput")
    gidx_d = nc.dram_tensor("gidx", [128, NCHT * 8], I16, kind="ExternalInput")
    ident_d = nc.dram_tensor("ident", [128, 128], BF16, kind="ExternalInput")
    ohs_d = nc.dram_tensor("ohs", [128, NCHT * 128], FP8, kind="ExternalInput")
    y = nc.dram_tensor("y", [NP, OUT], F32, kind="ExternalOutput")

    hs1_stage = nc.dram_tensor("hs1_stage", [NP, HID], BF16)
    hsr2_stage = nc.dram_tensor("hsr2_stage", [NP, 128], BF16)
    hs1_full = nc.dram_tensor("hs1_full", [N, HID], BF16, addr_space="Shared")
    hsr2_full = nc.dram_tensor("hsr2_full", [N, 128], BF16, addr_space="Shared")
    rg = [list(range(ncores))]
    qn = [0]

    def next_q():
        q = qn[0]
        qn[0] = (q + 1) % NQUEUES
        return q

    # last chunk of each block (end of its remote stream) -> stop flag
    lastc = {b: cbase[b, 0] + int(CH[b, 0]) - 1 for b in range(nblk)}
    chunk_blk = {}
    for b in range(nblk):
        for g in range(NS):
            for cc in range(int(CH[b, g])):
                chunk_blk[cbase[b, g] + cc] = b
    segv = {c0: nv for (_, c0, _), nv in zip(segments, seg_valid)}

    def remote_order():
        order = []
        for r in range(NR):
            order += [(c0, sch) for (c0, sch) in rem_segs[r]]
        return order

    XG = 10  # blocks per x-load slice

    with tile.TileContext(nc) as tc:
        with (
            tc.tile_pool(name="const", bufs=1) as constp,
            tc.tile_pool(name="hs", bufs=4) as hsp,
        ):
            nc.gpsimd.load_library(library_config.mlp)

            dinv_sb = constp.tile([128, nblk], F32, tag="dinv")
            b2_bc = constp.tile([128, OUT], F32, tag="b2bc")
            dinv_bc = constp.tile([128, nblk * BLK], F32, tag="dinvbc")
            gidx_sb = constp.tile([128, NCHT * 8], I16, tag="gidx")
            ident_sb = constp.tile([128, 128], BF16, tag="ident")
            w2_sb = constp.tile([HID, OUT], F32, tag="w2")
            b1_sb = constp.tile([HID, 1], F32, tag="b1")
            b2_sb = constp.tile([1, OUT], F32, tag="b2")
            ones_sb = constp.tile([1, 128], F32, tag="ones")
            ohs_sb = constp.tile([128, NCHT * 128], FP8, tag="ohs")
            hs1_t = []

            with tc.tile_pool(name="pre", bufs=1) as prep:
                # ---- loads needed by phase B ----
                xsb = {}
                for k in range(KC):
                    for s in range(0, nblk, XG):
                        cols = sum(widths[s : s + XG])
                        t = prep.tile([128, XG * BLK], BF16, tag=f"x{k}_{s}")
                        nc.sync.dma_start(
                            out=t[:, :cols],
                            in_=x_tr[k * 128 : (k + 1) * 128,
                                     s * BLK : s * BLK + cols],
                        )
                        xsb[k, s] = t
                w1c = []
                for k in range(KC):
                    t = prep.tile([128, HID], BF16, tag=f"w1c{k}")
                    nc.sync.dma_start(out=t[:], in_=w1[k * 128 : (k + 1) * 128, :])
                    w1c.append(t)
                nc.sync.dma_start(out=dinv_sb[:], in_=deg_own[:, :])
                nc.scalar.sqrt(dinv_sb[:], dinv_sb[:])
                nc.vector.reciprocal(dinv_sb[:], dinv_sb[:])

                # all remaining constants up-front on the sync HWDGE queue so
                # they never overlap (and starve) the first AllGather
                nc.sync.dma_start(out=gidx_sb[:], in_=gidx_d[:, :])
                nc.sync.dma_start(out=ident_sb[:], in_=ident_d[:, :])
                nc.sync.dma_start(out=w2_sb[:], in_=w2[:, :])
                nc.sync.dma_start(out=b1_sb[:], in_=b1c[:, :])
                nc.sync.dma_start(out=b2_sb[:], in_=b2[:, :])
                nc.vector.memset(ones_sb[:], 1.0)
                qcols = _cdiv(NCHT * 128, 4)
                for qq in range(4):
                    c0q = qq * qcols
                    c1q = min((qq + 1) * qcols, NCHT * 128)
                    nc.sync.dma_start(out=ohs_sb[:, c0q:c1q],
                                      in_=ohs_d[:, c0q:c1q])
                deg_rsb = prep.tile([1, nblk * BLK], F32, tag="degrow")
                nc.sync.dma_start(out=deg_rsb[:], in_=deg_row[:, :])

                # ---- phase B + broadcasts (own PSUM scope) ----
                with tc.tile_pool(name="psB", bufs=2, space="PSUM") as psB:
                    for b in range(nblk):
                        w = widths[b]
                        ph = psB.tile([128, HID], F32, tag="acc")
                        for k in range(KC):
                            nc.tensor.matmul(
                                ph[:w, :],
                                lhsT=xsb[k, (b // XG) * XG][
                                    :, (b % XG) * BLK : (b % XG) * BLK + w],
                                rhs=w1c[k][:, :],
                                start=(k == 0),
                                stop=(k == KC - 1),
                            )
                        t = constp.tile([128, HID], BF16, tag=f"hs1_{b}",
                                        name=f"hs1t_{b}")
                        nc.scalar.activation(
                            t[:w, :],
                            ph[:w, :],
                            mybir.ActivationFunctionType.Copy,
                            scale=dinv_sb[:w, b : b + 1],
                        )
                        nc.scalar.dma_start(
                            out=hs1_stage[b * BLK : b * BLK + w, :], in_=t[:w, :]
                        )
                        hs1_t.append(t)

                    # broadcast b2 to all partitions via rank-1 matmul
                    pb2 = psB.tile([128, 128], F32, tag="aux")
                    nc.tensor.matmul(pb2[:, :OUT], lhsT=ones_sb[:], rhs=b2_sb[:],
                                     start=True, stop=True)
                    nc.vector.tensor_copy(b2_bc[:], pb2[:, :OUT])

                    # per-column dinv for the transposed layer-1 epilogue
                    for b in range(nblk):
                        pdv = psB.tile([128, 128], F32, tag="aux")
                        nc.tensor.matmul(
                            pdv[:], lhsT=ones_sb[:],
                            rhs=deg_rsb[:, b * BLK : (b + 1) * BLK],
                            start=True, stop=True,
                        )
                        nc.vector.tensor_copy(
                            dinv_bc[:, b * BLK : (b + 1) * BLK], pdv[:])
                    nc.scalar.sqrt(dinv_bc[:], dinv_bc[:])
                    nc.vector.reciprocal(dinv_bc[:], dinv_bc[:])

            with (
                tc.tile_pool(name="gath", bufs=14) as gathp,
            ):
                # zero the gather pool once: trailing-trimmed lanes expose
                # stale SBUF; first use must not contain NaN-decoding garbage
                for zi in range(14):
                    zt = gathp.tile([128, MAXCH, HID], BF16, tag="g",
                                    name=f"z{zi}")
                    nc.vector.memset(zt[:, :, :], 0.0)

                def issue_gather(pool, tag, table, c0, sch, elem):
                    t = pool.tile([128, MAXCH, HID], BF16, tag=tag,
                                  name=f"gt{c0}")
                    nc.gpsimd.dma_gather(
                        t[:, :sch, :],
                        table.ap(),
                        gidx_sb[:, c0 * 8 : (c0 + sch) * 8],
                        sch * 128,
                        segv[c0],
                        elem,
                        queue_num=next_q(),
                        single_packet=False,
                    )
                    return t

                def one_mm(layer, acc, b, t, cc, gc, stop):
                    w = widths[b]
                    if layer == 1:
                        nc.tensor.matmul(
                            acc[b][:, :w],
                            lhsT=t[:, cc, :],
                            rhs=ohs_sb[:, gc * 128 : gc * 128 + w],
                            start=False,
                            stop=stop,
                        )
                    else:
                        nc.tensor.matmul(
                            acc[b][:w, :OUT],
                            lhsT=ohs_sb[:, gc * 128 : gc * 128 + w],
                            rhs=t[:, cc, :OUT],
                            start=False,
                            stop=stop,
                        )

                def run_layer(layer, stage_t, full_t, elem, acc_shape,
                              acc_pool, selfloop, epilogue, collective):
                    # the AllGather runs exclusively (concurrent SWDGE gather
                    # traffic throttles the CC DMA ~5x); triggered from the
                    # vector engine so it never occupies the gather stream
                    collective()
                    rorder = remote_order()
                    rem_tiles = {}
                    ri = [0]

                    def consume_upto(n):
                        while ri[0] < min(n, len(rorder)):
                            c0, sch = rorder[ri[0]]
                            rem_tiles[c0] = issue_gather(
                                gathp, "g", full_t, c0, sch, elem)
                            ri[0] += 1

                    marks = []
                    done = 0
                    for r in range(NR):
                        done += len(rem_segs[r])
                        nxt = len(rem_segs[r + 1]) if r + 1 < NR else 0
                        marks.append(done + nxt)

                    for r in range(NR):
                        consume_upto(marks[r])
                        acc_cur = {}
                        for b in rounds[r]:
                            acc_cur[b] = acc_pool.tile(
                                acc_shape, F32, tag=f"acc{b - r * R}",
                                name=f"acc{layer}_{b}",
                            )
                            selfloop(acc_cur, b)
                        for (c0, sch) in rem_segs[r]:
                            t = rem_tiles[c0]
                            for cc in range(sch):
                                gc = c0 + cc
                                b = chunk_blk[gc]
                                one_mm(layer, acc_cur, b, t, cc, gc,
                                       gc == lastc[b])
                        for b in rounds[r]:
                            epilogue(acc_cur, b)
                    consume_upto(len(rorder))

                # ---- layer 1: S1^T -> hsr^T -> hsr2 ----
                hsr2_t = [None] * nblk
                with tc.tile_pool(name="psD", bufs=1, space="PSUM") as psD:
                    p2s_bufs = [
                        psD.tile([128, 128], F32, tag=f"p2s{i}", name=f"p2s{i}")
                        for i in range(2)
                    ]
                    epi_i = [0]

                    def selfloop1(acc, b):
                        w = widths[b]
                        nc.tensor.matmul(
                            acc[b][:, :w], lhsT=hs1_t[b][:w, :],
                            rhs=ident_sb[:w, :w],
                            start=True, stop=False,
                        )

                    def epilogue1(acc, b):
                        w = widths[b]
                        t1 = hsp.tile([128, 128], F32, tag="t1", name=f"t1_{b}")
                        nc.vector.tensor_tensor(
                            out=t1[:, :w], in0=acc[b][:, :w],
                            in1=dinv_bc[:, b * BLK : b * BLK + w],
                            op=mybir.AluOpType.mult,
                        )
                        hsrT = hsp.tile([128, 128], F32, tag="hsrT",
                                        name=f"hsrT_{b}")
                        nc.scalar.activation(
                            hsrT[:, :w], t1[:, :w],
                            mybir.ActivationFunctionType.Relu,
                            bias=b1_sb[:, 0:1],
                        )
                        p2s = p2s_bufs[epi_i[0] % 2]
                        epi_i[0] += 1
                        nc.tensor.matmul(
                            p2s[:w, :OUT], lhsT=hsrT[:, :w], rhs=w2_sb[:, :],
                            start=True, stop=True,
                        )
                        t2 = constp.tile([128, 128], BF16, tag=f"hsr2_{b}",
                                         name=f"hsr2t_{b}")
                        nc.vector.memset(t2[:, OUT:], 0.0)
                        nc.scalar.activation(
                            t2[:w, :OUT], p2s[:w, :OUT],
                            mybir.ActivationFunctionType.Copy,
                            scale=dinv_sb[:w, b : b + 1],
                        )
                        nc.scalar.dma_start(
                            out=hsr2_stage[b * BLK : b * BLK + w, :],
                            in_=t2[:w, :],
                        )
                        hsr2_t[b] = t2

                    def ag1():
                        nc.gpsimd.collective_compute(
                            "AllGather",
                            mybir.AluOpType.bypass,
                            replica_groups=rg,
                            ins=[hs1_stage[0:NP, :].opt()],
                            outs=[hs1_full[0 : ncores * NP, :].opt()],
                        )

                    run_layer(1, hs1_stage, hs1_full, HID, [128, 128], psD,
                              selfloop1, epilogue1, ag1)

                # ---- layer 2: S2 -> y ----
                with tc.tile_pool(name="psF", bufs=1, space="PSUM") as psF:

                    def selfloop2(acc, b):
                        w = widths[b]
                        nc.tensor.matmul(
                            acc[b][:w, :OUT], lhsT=ident_sb[:w, :w],
                            rhs=hsr2_t[b][:w, :OUT],
                            start=True, stop=False,
                        )

                    def epilogue2(acc, b):
                        w = widths[b]
                        o1 = hsp.tile([128, OUT], F32, tag="o1", name=f"o1_{b}")
                        nc.scalar.activation(
                            o1[:w, :], acc[b][:w, :OUT],
                            mybir.ActivationFunctionType.Copy,
                            scale=dinv_sb[:w, b : b + 1],
                        )
                        yt = hsp.tile([128, OUT], F32, tag="yt", name=f"yt_{b}")
                        nc.vector.tensor_tensor(
                            out=yt[:w, :], in0=o1[:w, :], in1=b2_bc[:w, :],
                            op=mybir.AluOpType.add,
                        )
                        nc.scalar.dma_start(out=y[b * BLK : b * BLK + w, :],
                                              in_=yt[:w, :])

                    def ag2():
                        nc.gpsimd.collective_compute(
                            "AllGather",
                            mybir.AluOpType.bypass,
                            replica_groups=rg,
                            ins=[hsr2_stage[0:NP, :].opt()],
                            outs=[hsr2_full[0 : ncores * NP, :].opt()],
                        )

                    run_layer(2, hsr2_stage, hsr2_full, 128, [128, 64], psF,
                              selfloop2, epilogue2, ag2)

    nc.compile()
    return nc


def _make_ident():
    import ml_dtypes

    return np.eye(128, dtype=np.float32).astype(ml_dtypes.bfloat16)


_IDENT = _make_ident()


def make_in_maps(per_core, W1, b1, W2, b2):
    import ml_dtypes

    W1 = np.ascontiguousarray(np.asarray(W1, np.float32).astype(ml_dtypes.bfloat16))
    W2 = np.ascontiguousarray(np.asarray(W2, np.float32))
    b1 = np.ascontiguousarray(np.asarray(b1, np.float32).reshape(-1, 1))
    b2 = np.asarray(b2, np.float32).reshape(1, -1)
    return [
        {
            "x_tr": pc["x_tr"],
            "w1": W1,
            "b1c": b1,
            "w2": W2,
            "b2": b2,
            "deg_own": pc["deg_own"],
            "deg_row": pc["deg_row"],
            "gidx": pc["gidx"],
            "ohs": pc["ohs"],
            "ident": _IDENT,
        }
        for pc in per_core
    ]


def kernel_run(x, edge_index, W1, b1, W2, b2, trace=False, tmpdir=None):
    x = np.ascontiguousarray(np.asarray(x, np.float32))
    per_core, meta = preprocess(x, edge_index)
    HID = np.asarray(W1).shape[1]
    OUT = np.asarray(W2).shape[1]
    nc = build_nc(meta, HID, OUT)
    in_maps = make_in_maps(per_core, W1, b1, W2, b2)
    res = run_bass_kernel_spmd(
        nc, in_maps, core_ids=list(range(NCORES)), trace=trace, tmpdir=tmpdir
    )
    out = np.concatenate([r["y"] for r in res.results], axis=0)
    return out, res


def kernel(x, edge_index, W1, b1, W2, b2):
    out, _ = kernel_run(x, edge_index, W1, b1, W2, b2)
    return out


# revision 18
# speedup vs baseline: 1.3104x; 1.0549x over previous
"""2-layer GCN encoder as a distributed Bass kernel on 8 TRN2 NeuronCores.

Decomposition (per core, nodes sharded by destination):
  hs1[v]  = dinv[v] * (x[v] @ W1)                 (own rows, bf16)
  S1T[:,d]= sum_{e: dst=d} hs1[src_e]             (dma_gather + one-hot matmul,
                                                   accumulated TRANSPOSED)
  hsrT    = relu(dinv_col * S1T + b1)             (dst-side dinv per column)
  hsr2[v] = dinv[v] * (hsrT^T @ W2)               (W2 commutes with the layer-2
                                                   edge sum -> aggregate at 64)
  S2[d]   = sum_{e: dst=d} hsr2[src_e]
  y[d]    = dinv[d]*S2[d] + b2

The critical resource is SWDGE descriptor emission (~2ns of Q7 time per
gathered row, 4 queue-pairs).  Sources are split into three streams per
destination block:
  own  - sources owned by this core, gathered from the LOCAL stage tensor;
         no collective dependency, so these gathers run inside the CC-barrier
         / AllGather windows that would otherwise idle the Q7.
  g0   - remote sources from every rank's first GSPLIT blocks (AllGather 0)
  g1   - remote sources from the remaining blocks (AllGather 1)
Each layer's AllGather runs as two chunked collectives; layer-2's triggers
are placed after ALL of layer 1's gather instructions so they never stall
the in-order GpSimd stream.  Remote segments issue with g0 lookahead
([g0 r0][g0 r1][g0 r2][g1 r0][g0 r3][g1 r1]...) so g1 consumption starts
only after its AllGather has had time to complete.

Destination blocks are processed in rounds of 6 (PSUM allocations are
bank-granular: 6 accumulators + 2 aux = 8 banks).  Gather instructions pack
up to MAXCH=16 chunks across block boundaries (single_packet=False is
required above 64 descriptors per engine).  Per-segment valid counts are
uniform across cores (num_idxs_reg); interior pads gather row 0 against
zero one-hot columns, trailing pads are -1 on every core identically.
"""

import numpy as np

import concourse.bass as bass
import concourse.bacc as bacc
import concourse.mybir as mybir
import concourse.tile as tile
from concourse import library_config
from concourse.bass_utils import run_bass_kernel_spmd

F32 = mybir.dt.float32
BF16 = mybir.dt.bfloat16
FP8 = mybir.dt.float8e4
I16 = mybir.dt.int16

NCORES = 8
BLK = 128
MAXCH = 8      # chunks (128 idx each) per dma_gather instruction
NQUEUES = 4
NS = 1         # single source stream per block (full AllGather table)


def _cdiv(a, b):
    return (a + b - 1) // b


def preprocess(x, edge_index, ncores=NCORES):
    """Host-side graph partitioning: shard nodes/edges by dst, split sources
    into own/remote-g0/remote-g1 streams, build per-core gather indices and
    the one-hot chunk matrices (fp8)."""
    import ml_dtypes

    N, IN = x.shape
    assert N % ncores == 0
    NP = N // ncores
    nblk = _cdiv(NP, BLK)
    R = 6                              # blocks per processing round
    NR = _cdiv(nblk, R)
    widths = [min(BLK, NP - b * BLK) for b in range(nblk)]
    rounds = [list(range(r * R, min((r + 1) * R, nblk))) for r in range(NR)]

    src = np.asarray(edge_index[0], dtype=np.int64)
    dst = np.asarray(edge_index[1], dtype=np.int64)
    deg = (np.bincount(dst, minlength=N) + 1).astype(np.float32)

    # dedupe repeated (src, dst) pairs; multiplicity goes into the multi-hot
    key = dst * N + src
    ukey, mult = np.unique(key, return_counts=True)
    dst_s = ukey // N
    src_s = ukey % N
    mult = mult.astype(np.float32)

    srcr = src_s // NP
    srco = src_s % NP

    bounds = np.array(
        [i * NP + b * BLK for i in range(ncores) for b in range(nblk)] + [N],
        dtype=np.int64,
    )
    pos = np.searchsorted(dst_s, bounds)

    # per (core, block, stream): sorted unique rows + scatter triplets
    blk_rows = {}
    blk_scatter = {}
    ucnt = np.zeros((ncores, nblk, NS), np.int64)
    for i in range(ncores):
        for b in range(nblk):
            k = i * nblk + b
            s0, s1 = pos[k], pos[k + 1]
            dl = (dst_s[s0:s1] - (i * NP + b * BLK)).astype(np.int64)
            urows, inv = np.unique(src_s[s0:s1], return_inverse=True)
            ucnt[i, b, 0] = len(urows)
            blk_rows[i, b, 0] = urows
            blk_scatter[i, b, 0] = (inv, dl, mult[s0:s1])

    CH = np.maximum(1, _cdiv(ucnt.max(axis=0), 128)).astype(np.int64)  # [b, g]

    # chunk layout: plain block-major stream
    cbase = {}
    c = 0
    for b in range(nblk):
        cbase[b, 0] = c
        c += int(CH[b, 0])
    NCHT = c

    # segments pack MAXCH chunks across block boundaries
    rem_segs = [(c0, min(MAXCH, NCHT - c0)) for c0 in range(0, NCHT, MAXCH)]
    segments = [(0, c0, sch) for (c0, sch) in rem_segs]

    # first pass: raw index values per core (-1 where no real source)
    all_vals = []
    all_ohs = []
    for i in range(ncores):
        vals = np.full(NCHT * 128, -1, np.int64)
        ohs = np.zeros((128, NCHT * 128), np.float32)
        for b in range(nblk):
            for g in range(NS):
                urows = blk_rows[i, b, g]
                inv, dl, mlt = blk_scatter[i, b, g]
                c0 = cbase[b, g]
                vals[c0 * 128 : c0 * 128 + len(urows)] = urows
                gc = c0 + inv // 128
                p = inv % 128
                np.add.at(ohs, (p, gc * 128 + dl), mlt)
        all_vals.append(vals)
        all_ohs.append(ohs)

    # per-segment valid count must be uniform across cores: num_idxs_reg is
    # baked into the shared program, and the ucode's trailing trim must land
    # exactly at the register value on every core
    seg_valid = []
    for (_, c0, sch) in segments:
        nv = 1
        for i in range(ncores):
            v = all_vals[i][c0 * 128 : (c0 + sch) * 128]
            nz = np.nonzero(v >= 0)[0]
            if len(nz):
                nv = max(nv, int(nz[-1]) + 1)
        seg_valid.append(nv)

    per_core = []
    for i in range(ncores):
        vals = all_vals[i]
        gidx = np.zeros((128, NCHT * 8), np.int16)
        for si, (_, c0, sch) in enumerate(segments):
            v = vals[c0 * 128 : (c0 + sch) * 128].copy()
            nv = seg_valid[si]
            v[:nv][v[:nv] < 0] = 0     # interior pads gather row 0
            v[nv:] = -1                # uniform trailing trim point
            wr = v.reshape(sch * 8, 16).T
            gidx[:, c0 * 8 : (c0 + sch) * 8] = np.tile(wr.astype(np.int16), (8, 1))

        degp = np.concatenate(
            [deg[i * NP : (i + 1) * NP], np.ones(nblk * BLK - NP, np.float32)]
        )
        per_core.append(
            {
                "x_tr": np.ascontiguousarray(
                    x[i * NP : (i + 1) * NP].T.astype(ml_dtypes.bfloat16)
                ),
                "deg_own": np.ascontiguousarray(degp.reshape(nblk, BLK).T),
                "deg_row": np.ascontiguousarray(degp.reshape(1, nblk * BLK)),
                "gidx": gidx,
                "ohs": all_ohs[i].astype(ml_dtypes.float8_e4m3),
            }
        )

    meta = {
        "N": N,
        "NP": NP,
        "IN": IN,
        "nblk": nblk,
        "R": R,
        "NR": NR,
        "rounds": rounds,
        "widths": widths,
        "CH": CH,
        "cbase": cbase,
        "rem_segs": rem_segs,
        "segments": segments,
        "seg_valid": seg_valid,
        "NCHT": NCHT,
    }
    return per_core, meta


def build_nc(meta, HID, OUT, ncores=NCORES):
    N, NP, IN = meta["N"], meta["NP"], meta["IN"]
    nblk, widths = meta["nblk"], meta["widths"]
    R, NR, rounds = meta["R"], meta["NR"], meta["rounds"]
    CH, cbase, NCHT = meta["CH"], meta["cbase"], meta["NCHT"]
    segments, seg_valid = meta["segments"], meta["seg_valid"]
    rem_segs = meta["rem_segs"]
    KC = IN // 128
    assert IN % 128 == 0 and HID == 128 and OUT <= 128

    nc = bacc.Bacc(
        "TRN2",
        target_bir_lowering=False,
        debug=False,
        num_devices=ncores,
        num_swdge_queues=NQUEUES,
    )

    x_tr = nc.dram_tensor("x_tr", [IN, NP], BF16, kind="ExternalInput")
    w1 = nc.dram_tensor("w1", [IN, HID], BF16, kind="ExternalInput")
    b1c = nc.dram_tensor("b1c", [HID, 1], F32, kind="ExternalInput")
    w2 = nc.dram_tensor("w2", [HID, OUT], F32, kind="ExternalInput")
    b2 = nc.dram_tensor("b2", [1, OUT], F32, kind="ExternalInput")
    deg_own = nc.dram_tensor("deg_own", [128, nblk], F32, kind="ExternalInput")
    deg_row = nc.dram_tensor("deg_row", [1, nblk * BLK], F32, kind="ExternalInput")
    gidx_d = nc.dram_tensor("gidx", [128, NCHT * 8], I16, kind="ExternalInput")
    ident_d = nc.dram_tensor("ident", [128, 128], BF16, kind="ExternalInput")
    ohs_d = nc.dram_tensor("ohs", [128, NCHT * 128], FP8, kind="ExternalInput")
    y = nc.dram_tensor("y", [NP, OUT], F32, kind="ExternalOutput")

    hs1_stage = nc.dram_tensor("hs1_stage", [NP, HID], BF16)
    hsr2_stage = nc.dram_tensor("hsr2_stage", [NP, 128], BF16)
    hs1_full = nc.dram_tensor("hs1_full", [N, HID], BF16, addr_space="Shared")
    hsr2_full = nc.dram_tensor("hsr2_full", [N, 128], BF16, addr_space="Shared")
    rg = [list(range(ncores))]
    qn = [0]

    def next_q():
        q = qn[0]
        qn[0] = (q + 1) % NQUEUES
        return q

    # last chunk of each block (end of its remote stream) -> stop flag
    lastc = {b: cbase[b, 0] + int(CH[b, 0]) - 1 for b in range(nblk)}
    chunk_blk = {}
    for b in range(nblk):
        for g in range(NS):
            for cc in range(int(CH[b, g])):
                chunk_blk[cbase[b, g] + cc] = b
    segv = {c0: nv for (_, c0, _), nv in zip(segments, seg_valid)}

    def remote_order():
        order = []
        for r in range(NR):
            order += [(c0, sch) for (c0, sch) in rem_segs[r]]
        return order

    XG = 10  # blocks per x-load slice

    with tile.TileContext(nc) as tc:
        with (
            tc.tile_pool(name="const", bufs=1) as constp,
            tc.tile_pool(name="hs", bufs=4) as hsp,
        ):
            nc.gpsimd.load_library(library_config.mlp)

            dinv_sb = constp.tile([128, nblk], F32, tag="dinv")
            b2_bc = constp.tile([128, OUT], F32, tag="b2bc")
            dinv_bc = constp.tile([128, nblk * BLK], F32, tag="dinvbc")
            gidx_sb = constp.tile([128, NCHT * 8], I16, tag="gidx")
            ident_sb = constp.tile([128, 128], BF16, tag="ident")
            w2_sb = constp.tile([HID, OUT], F32, tag="w2")
            b1_sb = constp.tile([HID, 1], F32, tag="b1")
            b2_sb = constp.tile([1, OUT], F32, tag="b2")
            ones_sb = constp.tile([1, 128], F32, tag="ones")
            ohs_sb = constp.tile([128, NCHT * 128], FP8, tag="ohs")
            hs1_t = []

            with tc.tile_pool(name="pre", bufs=1) as prep:
                # ---- loads needed by phase B ----
                xsb = {}
                for k in range(KC):
                    for s in range(0, nblk, XG):
                        cols = sum(widths[s : s + XG])
                        t = prep.tile([128, XG * BLK], BF16, tag=f"x{k}_{s}")
                        nc.sync.dma_start(
                            out=t[:, :cols],
                            in_=x_tr[k * 128 : (k + 1) * 128,
                                     s * BLK : s * BLK + cols],
                        )
                        xsb[k, s] = t
                w1c = []
                for k in range(KC):
                    t = prep.tile([128, HID], BF16, tag=f"w1c{k}")
                    nc.sync.dma_start(out=t[:], in_=w1[k * 128 : (k + 1) * 128, :])
                    w1c.append(t)
                nc.sync.dma_start(out=dinv_sb[:], in_=deg_own[:, :])
                nc.scalar.sqrt(dinv_sb[:], dinv_sb[:])
                nc.vector.reciprocal(dinv_sb[:], dinv_sb[:])

                # all remaining constants up-front on the sync HWDGE queue so
                # they never overlap (and starve) the first AllGather
                nc.sync.dma_start(out=gidx_sb[:], in_=gidx_d[:, :])
                nc.sync.dma_start(out=ident_sb[:], in_=ident_d[:, :])
                nc.sync.dma_start(out=w2_sb[:], in_=w2[:, :])
                nc.sync.dma_start(out=b1_sb[:], in_=b1c[:, :])
                nc.sync.dma_start(out=b2_sb[:], in_=b2[:, :])
                nc.vector.memset(ones_sb[:], 1.0)
                qcols = _cdiv(NCHT * 128, 4)
                for qq in range(4):
                    c0q = qq * qcols
                    c1q = min((qq + 1) * qcols, NCHT * 128)
                    nc.sync.dma_start(out=ohs_sb[:, c0q:c1q],
                                      in_=ohs_d[:, c0q:c1q])
                deg_rsb = prep.tile([1, nblk * BLK], F32, tag="degrow")
                nc.sync.dma_start(out=deg_rsb[:], in_=deg_row[:, :])

                # ---- phase B + broadcasts (own PSUM scope) ----
                with tc.tile_pool(name="psB", bufs=2, space="PSUM") as psB:
                    for b in range(nblk):
                        w = widths[b]
                        ph = psB.tile([128, HID], F32, tag="acc")
                        for k in range(KC):
                            nc.tensor.matmul(
                                ph[:w, :],
                                lhsT=xsb[k, (b // XG) * XG][
                                    :, (b % XG) * BLK : (b % XG) * BLK + w],
                                rhs=w1c[k][:, :],
                                start=(k == 0),
                                stop=(k == KC - 1),
                            )
                        t = constp.tile([128, HID], BF16, tag=f"hs1_{b}",
                                        name=f"hs1t_{b}")
                        nc.scalar.activation(
                            t[:w, :],
                            ph[:w, :],
                            mybir.ActivationFunctionType.Copy,
                            scale=dinv_sb[:w, b : b + 1],
                        )
                        nc.scalar.dma_start(
                            out=hs1_stage[b * BLK : b * BLK + w, :], in_=t[:w, :]
                        )
                        hs1_t.append(t)

                    # broadcast b2 to all partitions via rank-1 matmul
                    pb2 = psB.tile([128, 128], F32, tag="aux")
                    nc.tensor.matmul(pb2[:, :OUT], lhsT=ones_sb[:], rhs=b2_sb[:],
                                     start=True, stop=True)
                    nc.vector.tensor_copy(b2_bc[:], pb2[:, :OUT])

                    # per-column dinv for the transposed layer-1 epilogue
                    for b in range(nblk):
                        pdv = psB.tile([128, 128], F32, tag="aux")
                        nc.tensor.matmul(
                            pdv[:], lhsT=ones_sb[:],
                            rhs=deg_rsb[:, b * BLK : (b + 1) * BLK],
                            start=True, stop=True,
                        )
                        nc.vector.tensor_copy(
                            dinv_bc[:, b * BLK : (b + 1) * BLK], pdv[:])
                    nc.scalar.sqrt(dinv_bc[:], dinv_bc[:])
                    nc.vector.reciprocal(dinv_bc[:], dinv_bc[:])

            with (
                tc.tile_pool(name="gath", bufs=16) as gathp,
            ):
                # zero the gather pool once: trailing-trimmed lanes expose
                # stale SBUF; first use must not contain NaN-decoding garbage
                for zi in range(16):
                    zt = gathp.tile([128, MAXCH, HID], BF16, tag="g",
                                    name=f"z{zi}")
                    nc.vector.memset(zt[:, :, :], 0.0)

                def issue_gather(pool, tag, table, c0, sch, elem):
                    t = pool.tile([128, MAXCH, HID], BF16, tag=tag,
                                  name=f"gt{c0}")
                    nc.gpsimd.dma_gather(
                        t[:, :sch, :],
                        table.ap(),
                        gidx_sb[:, c0 * 8 : (c0 + sch) * 8],
                        sch * 128,
                        segv[c0],
                        elem,
                        queue_num=next_q(),
                        single_packet=False,
                    )
                    return t

                def one_mm(layer, acc, b, t, cc, gc, stop):
                    w = widths[b]
                    if layer == 1:
                        nc.tensor.matmul(
                            acc[b][:, :w],
                            lhsT=t[:, cc, :],
                            rhs=ohs_sb[:, gc * 128 : gc * 128 + w],
                            start=False,
                            stop=stop,
                        )
                    else:
                        nc.tensor.matmul(
                            acc[b][:w, :OUT],
                            lhsT=ohs_sb[:, gc * 128 : gc * 128 + w],
                            rhs=t[:, cc, :OUT],
                            start=False,
                            stop=stop,
                        )

                def run_layer(layer, stage_t, full_t, elem, acc_shape,
                              acc_pool, selfloop, epilogue, collective):
                    # the AllGather runs exclusively (concurrent SWDGE gather
                    # traffic throttles the CC DMA ~5x)
                    collective()
                    rem_tiles = {}
                    ri = [0]
                    LOOKAHEAD = 6

                    def issue_upto(n):
                        while ri[0] < min(n, len(rem_segs)):
                            c0, sch = rem_segs[ri[0]]
                            rem_tiles[ri[0]] = issue_gather(
                                gathp, "g", full_t, c0, sch, elem)
                            ri[0] += 1

                    # fine-grained block-major pipeline: per-block epilogues
                    # flow continuously; gathers stay LOOKAHEAD segs ahead
                    acc_cur = {}
                    for b in range(nblk):
                        issue_upto((cbase[b, 0] + int(CH[b, 0])) // MAXCH
                                   + LOOKAHEAD)
                        acc_cur[b] = acc_pool.tile(
                            acc_shape, F32, tag=f"acc{b % 4}",
                            name=f"acc{layer}_{b}",
                        )
                        selfloop(acc_cur, b)
                        for gc in range(cbase[b, 0], cbase[b, 0] + int(CH[b, 0])):
                            si = gc // MAXCH
                            t = rem_tiles[si]
                            one_mm(layer, acc_cur, b, t, gc - si * MAXCH, gc,
                                   gc == lastc[b])
                        epilogue(acc_cur, b)
                    issue_upto(len(rem_segs))

                # ---- layer 1: S1^T -> hsr^T -> hsr2 ----
                hsr2_t = [None] * nblk
                with tc.tile_pool(name="psD", bufs=1, space="PSUM") as psD:
                    p2s_bufs = [
                        psD.tile([128, 128], F32, tag=f"p2s{i}", name=f"p2s{i}")
                        for i in range(2)
                    ]
                    epi_i = [0]

                    def selfloop1(acc, b):
                        w = widths[b]
                        nc.tensor.matmul(
                            acc[b][:, :w], lhsT=hs1_t[b][:w, :],
                            rhs=ident_sb[:w, :w],
                            start=True, stop=False,
                        )

                    def epilogue1(acc, b):
                        w = widths[b]
                        t1 = hsp.tile([128, 128], F32, tag="t1", name=f"t1_{b}")
                        nc.vector.tensor_tensor(
                            out=t1[:, :w], in0=acc[b][:, :w],
                            in1=dinv_bc[:, b * BLK : b * BLK + w],
                            op=mybir.AluOpType.mult,
                        )
                        hsrT = hsp.tile([128, 128], F32, tag="hsrT",
                                        name=f"hsrT_{b}")
                        nc.scalar.activation(
                            hsrT[:, :w], t1[:, :w],
                            mybir.ActivationFunctionType.Relu,
                            bias=b1_sb[:, 0:1],
                        )
                        p2s = p2s_bufs[epi_i[0] % 2]
                        epi_i[0] += 1
                        nc.tensor.matmul(
                            p2s[:w, :OUT], lhsT=hsrT[:, :w], rhs=w2_sb[:, :],
                            start=True, stop=True,
                        )
                        t2 = constp.tile([128, 128], BF16, tag=f"hsr2_{b}",
                                         name=f"hsr2t_{b}")
                        nc.vector.memset(t2[:, OUT:], 0.0)
                        nc.scalar.activation(
                            t2[:w, :OUT], p2s[:w, :OUT],
                            mybir.ActivationFunctionType.Copy,
                            scale=dinv_sb[:w, b : b + 1],
                        )
                        nc.scalar.dma_start(
                            out=hsr2_stage[b * BLK : b * BLK + w, :],
                            in_=t2[:w, :],
                        )
                        hsr2_t[b] = t2

                    def ag1():
                        nc.gpsimd.collective_compute(
                            "AllGather",
                            mybir.AluOpType.bypass,
                            replica_groups=rg,
                            ins=[hs1_stage[0:NP, :].opt()],
                            outs=[hs1_full[0 : ncores * NP, :].opt()],
                        )

                    run_layer(1, hs1_stage, hs1_full, HID, [128, 128], psD,
                              selfloop1, epilogue1, ag1)

                # ---- layer 2: S2 -> y ----
                with tc.tile_pool(name="psF", bufs=1, space="PSUM") as psF:

                    def selfloop2(acc, b):
                        w = widths[b]
                        nc.tensor.matmul(
                            acc[b][:w, :OUT], lhsT=ident_sb[:w, :w],
                            rhs=hsr2_t[b][:w, :OUT],
                            start=True, stop=False,
                        )

                    def epilogue2(acc, b):
                        w = widths[b]
                        o1 = hsp.tile([128, OUT], F32, tag="o1", name=f"o1_{b}")
                        nc.scalar.activation(
                            o1[:w, :], acc[b][:w, :OUT],
                            mybir.ActivationFunctionType.Copy,
                            scale=dinv_sb[:w, b : b + 1],
                        )
                        yt = hsp.tile([128, OUT], F32, tag="yt", name=f"yt_{b}")
                        nc.vector.tensor_tensor(
                            out=yt[:w, :], in0=o1[:w, :], in1=b2_bc[:w, :],
                            op=mybir.AluOpType.add,
                        )
                        nc.scalar.dma_start(out=y[b * BLK : b * BLK + w, :],
                                              in_=yt[:w, :])

                    def ag2():
                        nc.gpsimd.collective_compute(
                            "AllGather",
                            mybir.AluOpType.bypass,
                            replica_groups=rg,
                            ins=[hsr2_stage[0:NP, :].opt()],
                            outs=[hsr2_full[0 : ncores * NP, :].opt()],
                        )

                    run_layer(2, hsr2_stage, hsr2_full, 128, [128, 64], psF,
                              selfloop2, epilogue2, ag2)

    nc.compile()
    return nc


def _make_ident():
    import ml_dtypes

    return np.eye(128, dtype=np.float32).astype(ml_dtypes.bfloat16)


_IDENT = _make_ident()


def make_in_maps(per_core, W1, b1, W2, b2):
    import ml_dtypes

    W1 = np.ascontiguousarray(np.asarray(W1, np.float32).astype(ml_dtypes.bfloat16))
    W2 = np.ascontiguousarray(np.asarray(W2, np.float32))
    b1 = np.ascontiguousarray(np.asarray(b1, np.float32).reshape(-1, 1))
    b2 = np.asarray(b2, np.float32).reshape(1, -1)
    return [
        {
            "x_tr": pc["x_tr"],
            "w1": W1,
            "b1c": b1,
            "w2": W2,
            "b2": b2,
            "deg_own": pc["deg_own"],
            "deg_row": pc["deg_row"],
            "gidx": pc["gidx"],
            "ohs": pc["ohs"],
            "ident": _IDENT,
        }
        for pc in per_core
    ]


def kernel_run(x, edge_index, W1, b1, W2, b2, trace=False, tmpdir=None):
    x = np.ascontiguousarray(np.asarray(x, np.float32))
    per_core, meta = preprocess(x, edge_index)
    HID = np.asarray(W1).shape[1]
    OUT = np.asarray(W2).shape[1]
    nc = build_nc(meta, HID, OUT)
    in_maps = make_in_maps(per_core, W1, b1, W2, b2)
    res = run_bass_kernel_spmd(
        nc, in_maps, core_ids=list(range(NCORES)), trace=trace, tmpdir=tmpdir
    )
    out = np.concatenate([r["y"] for r in res.results], axis=0)
    return out, res


def kernel(x, edge_index, W1, b1, W2, b2):
    out, _ = kernel_run(x, edge_index, W1, b1, W2, b2)
    return out


# revision 20
# speedup vs baseline: 1.4512x; 1.1075x over previous
"""2-layer GCN encoder as a distributed Bass kernel on 8 TRN2 NeuronCores.

Decomposition (per core, nodes sharded by destination):
  hs1[v]  = dinv[v] * (x[v] @ W1)                 (own rows, bf16)
  S1T[:,d]= sum_{e: dst=d} hs1[src_e]             (dma_gather + one-hot matmul,
                                                   accumulated TRANSPOSED)
  hsrT    = relu(dinv_col * S1T + b1)             (dst-side dinv per column)
  hsr2[v] = dinv[v] * (hsrT^T @ W2)               (W2 commutes with the layer-2
                                                   edge sum -> aggregate at 64)
  S2[d]   = sum_{e: dst=d} hsr2[src_e]
  y[d]    = dinv[d]*S2[d] + b2

The critical resource is SWDGE descriptor emission (~2ns of Q7 time per
gathered row, 4 queue-pairs).  Sources are split into three streams per
destination block:
  own  - sources owned by this core, gathered from the LOCAL stage tensor;
         no collective dependency, so these gathers run inside the CC-barrier
         / AllGather windows that would otherwise idle the Q7.
  g0   - remote sources from every rank's first GSPLIT blocks (AllGather 0)
  g1   - remote sources from the remaining blocks (AllGather 1)
Each layer's AllGather runs as two chunked collectives; layer-2's triggers
are placed after ALL of layer 1's gather instructions so they never stall
the in-order GpSimd stream.  Remote segments issue with g0 lookahead
([g0 r0][g0 r1][g0 r2][g1 r0][g0 r3][g1 r1]...) so g1 consumption starts
only after its AllGather has had time to complete.

Destination blocks are processed in rounds of 6 (PSUM allocations are
bank-granular: 6 accumulators + 2 aux = 8 banks).  Gather instructions pack
up to MAXCH=16 chunks across block boundaries (single_packet=False is
required above 64 descriptors per engine).  Per-segment valid counts are
uniform across cores (num_idxs_reg); interior pads gather row 0 against
zero one-hot columns, trailing pads are -1 on every core identically.
"""

import numpy as np

import concourse.bass as bass
import concourse.bacc as bacc
import concourse.mybir as mybir
import concourse.tile as tile
from concourse import library_config
from concourse.bass_utils import run_bass_kernel_spmd

F32 = mybir.dt.float32
BF16 = mybir.dt.bfloat16
FP8 = mybir.dt.float8e4
I16 = mybir.dt.int16

NCORES = 8
BLK = 128
MAXCH = 8      # chunks (128 idx each) per dma_gather instruction
NQUEUES = 4
NS = 1         # single source stream per block (full AllGather table)


def _cdiv(a, b):
    return (a + b - 1) // b


def preprocess(x, edge_index, ncores=NCORES):
    """Host-side graph partitioning: shard nodes/edges by dst, split sources
    into own/remote-g0/remote-g1 streams, build per-core gather indices and
    the one-hot chunk matrices (fp8)."""
    import ml_dtypes

    N, IN = x.shape
    assert N % ncores == 0
    NP = N // ncores
    nblk = _cdiv(NP, BLK)
    R = 6                              # blocks per processing round
    NR = _cdiv(nblk, R)
    widths = [min(BLK, NP - b * BLK) for b in range(nblk)]
    rounds = [list(range(r * R, min((r + 1) * R, nblk))) for r in range(NR)]

    src = np.asarray(edge_index[0], dtype=np.int64)
    dst = np.asarray(edge_index[1], dtype=np.int64)
    deg = (np.bincount(dst, minlength=N) + 1).astype(np.float32)

    # dedupe repeated (src, dst) pairs; multiplicity goes into the multi-hot
    key = dst * N + src
    ukey, mult = np.unique(key, return_counts=True)
    dst_s = ukey // N
    src_s = ukey % N
    mult = mult.astype(np.float32)

    srcr = src_s // NP
    srco = src_s % NP

    bounds = np.array(
        [i * NP + b * BLK for i in range(ncores) for b in range(nblk)] + [N],
        dtype=np.int64,
    )
    pos = np.searchsorted(dst_s, bounds)

    # per (core, block, stream): sorted unique rows + scatter triplets
    blk_rows = {}
    blk_scatter = {}
    ucnt = np.zeros((ncores, nblk, NS), np.int64)
    for i in range(ncores):
        for b in range(nblk):
            k = i * nblk + b
            s0, s1 = pos[k], pos[k + 1]
            dl = (dst_s[s0:s1] - (i * NP + b * BLK)).astype(np.int64)
            # table row id in the (rank, partition, block) staging layout
            trow = srcr[s0:s1] * (nblk * BLK) + (srco[s0:s1] % BLK) * nblk \
                + srco[s0:s1] // BLK
            urows, inv = np.unique(trow, return_inverse=True)
            ucnt[i, b, 0] = len(urows)
            blk_rows[i, b, 0] = urows
            blk_scatter[i, b, 0] = (inv, dl, mult[s0:s1])

    CH = np.maximum(1, _cdiv(ucnt.max(axis=0), 128)).astype(np.int64)  # [b, g]

    # chunk layout: plain block-major stream
    cbase = {}
    c = 0
    for b in range(nblk):
        cbase[b, 0] = c
        c += int(CH[b, 0])
    NCHT = c

    # segments pack MAXCH chunks across block boundaries
    rem_segs = [(c0, min(MAXCH, NCHT - c0)) for c0 in range(0, NCHT, MAXCH)]
    segments = [(0, c0, sch) for (c0, sch) in rem_segs]

    # first pass: raw index values per core (-1 where no real source)
    all_vals = []
    all_ohs = []
    for i in range(ncores):
        vals = np.full(NCHT * 128, -1, np.int64)
        ohs = np.zeros((128, NCHT * 128), np.float32)
        for b in range(nblk):
            for g in range(NS):
                urows = blk_rows[i, b, g]
                inv, dl, mlt = blk_scatter[i, b, g]
                c0 = cbase[b, g]
                vals[c0 * 128 : c0 * 128 + len(urows)] = urows
                gc = c0 + inv // 128
                p = inv % 128
                np.add.at(ohs, (p, gc * 128 + dl), mlt)
        all_vals.append(vals)
        all_ohs.append(ohs)

    # per-segment valid count must be uniform across cores: num_idxs_reg is
    # baked into the shared program, and the ucode's trailing trim must land
    # exactly at the register value on every core
    seg_valid = []
    for (_, c0, sch) in segments:
        nv = 1
        for i in range(ncores):
            v = all_vals[i][c0 * 128 : (c0 + sch) * 128]
            nz = np.nonzero(v >= 0)[0]
            if len(nz):
                nv = max(nv, int(nz[-1]) + 1)
        seg_valid.append(nv)

    per_core = []
    for i in range(ncores):
        vals = all_vals[i]
        gidx = np.zeros((128, NCHT * 8), np.int16)
        for si, (_, c0, sch) in enumerate(segments):
            v = vals[c0 * 128 : (c0 + sch) * 128].copy()
            nv = seg_valid[si]
            v[:nv][v[:nv] < 0] = 0     # interior pads gather row 0
            v[nv:] = -1                # uniform trailing trim point
            wr = v.reshape(sch * 8, 16).T
            gidx[:, c0 * 8 : (c0 + sch) * 8] = np.tile(wr.astype(np.int16), (8, 1))

        degp = np.concatenate(
            [deg[i * NP : (i + 1) * NP], np.ones(nblk * BLK - NP, np.float32)]
        )
        per_core.append(
            {
                "x_tr": np.ascontiguousarray(
                    x[i * NP : (i + 1) * NP].T.astype(ml_dtypes.bfloat16)
                ),
                "deg_own": np.ascontiguousarray(degp.reshape(nblk, BLK).T),
                "deg_row": np.ascontiguousarray(degp.reshape(1, nblk * BLK)),
                "gidx": gidx,
                "ohs": all_ohs[i].astype(ml_dtypes.float8_e4m3),
            }
        )

    meta = {
        "N": N,
        "NP": NP,
        "IN": IN,
        "nblk": nblk,
        "R": R,
        "NR": NR,
        "rounds": rounds,
        "widths": widths,
        "CH": CH,
        "cbase": cbase,
        "rem_segs": rem_segs,
        "segments": segments,
        "seg_valid": seg_valid,
        "NCHT": NCHT,
    }
    return per_core, meta


def build_nc(meta, HID, OUT, ncores=NCORES):
    N, NP, IN = meta["N"], meta["NP"], meta["IN"]
    nblk, widths = meta["nblk"], meta["widths"]
    R, NR, rounds = meta["R"], meta["NR"], meta["rounds"]
    CH, cbase, NCHT = meta["CH"], meta["cbase"], meta["NCHT"]
    segments, seg_valid = meta["segments"], meta["seg_valid"]
    rem_segs = meta["rem_segs"]
    KC = IN // 128
    assert IN % 128 == 0 and HID == 128 and OUT <= 128

    nc = bacc.Bacc(
        "TRN2",
        target_bir_lowering=False,
        debug=False,
        num_devices=ncores,
        num_swdge_queues=NQUEUES,
    )

    x_tr = nc.dram_tensor("x_tr", [IN, NP], BF16, kind="ExternalInput")
    w1 = nc.dram_tensor("w1", [IN, HID], BF16, kind="ExternalInput")
    b1c = nc.dram_tensor("b1c", [HID, 1], F32, kind="ExternalInput")
    w2 = nc.dram_tensor("w2", [HID, OUT], F32, kind="ExternalInput")
    b2 = nc.dram_tensor("b2", [1, OUT], F32, kind="ExternalInput")
    deg_own = nc.dram_tensor("deg_own", [128, nblk], F32, kind="ExternalInput")
    deg_row = nc.dram_tensor("deg_row", [1, nblk * BLK], F32, kind="ExternalInput")
    gidx_d = nc.dram_tensor("gidx", [128, NCHT * 8], I16, kind="ExternalInput")
    ident_d = nc.dram_tensor("ident", [128, 128], BF16, kind="ExternalInput")
    ohs_d = nc.dram_tensor("ohs", [128, NCHT * 128], FP8, kind="ExternalInput")
    y = nc.dram_tensor("y", [NP, OUT], F32, kind="ExternalOutput")

    NPS = nblk * BLK  # padded rows per rank in the staging layout
    hs1_stage = nc.dram_tensor("hs1_stage", [NPS, HID], BF16)
    hsr2_stage = nc.dram_tensor("hsr2_stage", [NPS, 128], BF16)
    hs1_full = nc.dram_tensor("hs1_full", [ncores * NPS, HID], BF16,
                              addr_space="Shared")
    hsr2_full = nc.dram_tensor("hsr2_full", [ncores * NPS, 128], BF16,
                               addr_space="Shared")
    rg = [list(range(ncores))]
    qn = [0]

    def next_q():
        q = qn[0]
        qn[0] = (q + 1) % NQUEUES
        return q

    # last chunk of each block (end of its remote stream) -> stop flag
    lastc = {b: cbase[b, 0] + int(CH[b, 0]) - 1 for b in range(nblk)}
    chunk_blk = {}
    for b in range(nblk):
        for g in range(NS):
            for cc in range(int(CH[b, g])):
                chunk_blk[cbase[b, g] + cc] = b
    segv = {c0: nv for (_, c0, _), nv in zip(segments, seg_valid)}

    def remote_order():
        order = []
        for r in range(NR):
            order += [(c0, sch) for (c0, sch) in rem_segs[r]]
        return order

    XG = 10  # blocks per x-load slice

    with tile.TileContext(nc) as tc:
        with (
            tc.tile_pool(name="const", bufs=1) as constp,
            tc.tile_pool(name="hs", bufs=4) as hsp,
        ):
            nc.gpsimd.load_library(library_config.mlp)

            dinv_sb = constp.tile([128, nblk], F32, tag="dinv")
            b2_bc = constp.tile([128, OUT], F32, tag="b2bc")
            dinv_bc = constp.tile([128, nblk * BLK], F32, tag="dinvbc")
            gidx_sb = constp.tile([128, NCHT * 8], I16, tag="gidx")
            ident_sb = constp.tile([128, 128], BF16, tag="ident")
            w2_sb = constp.tile([HID, OUT], F32, tag="w2")
            b1_sb = constp.tile([HID, 1], F32, tag="b1")
            b2_sb = constp.tile([1, OUT], F32, tag="b2")
            ones_sb = constp.tile([1, 128], F32, tag="ones")
            ohs_sb = constp.tile([128, NCHT * 128], FP8, tag="ohs")
            hs1_t = []

            with tc.tile_pool(name="pre", bufs=1) as prep:
                # ---- loads needed by phase B ----
                xsb = {}
                for k in range(KC):
                    for s in range(0, nblk, XG):
                        cols = sum(widths[s : s + XG])
                        t = prep.tile([128, XG * BLK], BF16, tag=f"x{k}_{s}")
                        nc.sync.dma_start(
                            out=t[:, :cols],
                            in_=x_tr[k * 128 : (k + 1) * 128,
                                     s * BLK : s * BLK + cols],
                        )
                        xsb[k, s] = t
                w1c = []
                for k in range(KC):
                    t = prep.tile([128, HID], BF16, tag=f"w1c{k}")
                    nc.sync.dma_start(out=t[:], in_=w1[k * 128 : (k + 1) * 128, :])
                    w1c.append(t)
                nc.sync.dma_start(out=dinv_sb[:], in_=deg_own[:, :])
                nc.scalar.sqrt(dinv_sb[:], dinv_sb[:])
                nc.vector.reciprocal(dinv_sb[:], dinv_sb[:])

                # all remaining constants up-front on the sync HWDGE queue so
                # they never overlap (and starve) the first AllGather
                nc.sync.dma_start(out=gidx_sb[:], in_=gidx_d[:, :])
                nc.sync.dma_start(out=ident_sb[:], in_=ident_d[:, :])
                nc.sync.dma_start(out=w2_sb[:], in_=w2[:, :])
                nc.sync.dma_start(out=b1_sb[:], in_=b1c[:, :])
                nc.sync.dma_start(out=b2_sb[:], in_=b2[:, :])
                nc.vector.memset(ones_sb[:], 1.0)
                qcols = _cdiv(NCHT * 128, 4)
                for qq in range(4):
                    c0q = qq * qcols
                    c1q = min((qq + 1) * qcols, NCHT * 128)
                    nc.sync.dma_start(out=ohs_sb[:, c0q:c1q],
                                      in_=ohs_d[:, c0q:c1q])
                deg_rsb = prep.tile([1, nblk * BLK], F32, tag="degrow")
                nc.sync.dma_start(out=deg_rsb[:], in_=deg_row[:, :])

                # ---- phase B + broadcasts (own PSUM scope) ----
                hs1_big = constp.tile([128, nblk, HID], BF16, tag="hs1big")
                with tc.tile_pool(name="psB", bufs=2, space="PSUM") as psB:
                    for b in range(nblk):
                        w = widths[b]
                        ph = psB.tile([128, HID], F32, tag="acc")
                        for k in range(KC):
                            nc.tensor.matmul(
                                ph[:w, :],
                                lhsT=xsb[k, (b // XG) * XG][
                                    :, (b % XG) * BLK : (b % XG) * BLK + w],
                                rhs=w1c[k][:, :],
                                start=(k == 0),
                                stop=(k == KC - 1),
                            )
                        nc.scalar.activation(
                            hs1_big[:w, b, :],
                            ph[:w, :],
                            mybir.ActivationFunctionType.Copy,
                            scale=dinv_sb[:w, b : b + 1],
                        )
                        hs1_t.append(hs1_big)
                    # one bulk stage write in the (p, b) layout; pad rows
                    # carry garbage but are never indexed
                    nc.scalar.dma_start(
                        out=hs1_stage[0:NPS, :], in_=hs1_big[:, :, :],
                    )

                    # broadcast b2 to all partitions via rank-1 matmul
                    pb2 = psB.tile([128, 128], F32, tag="aux")
                    nc.tensor.matmul(pb2[:, :OUT], lhsT=ones_sb[:], rhs=b2_sb[:],
                                     start=True, stop=True)
                    nc.vector.tensor_copy(b2_bc[:], pb2[:, :OUT])

                    # per-column dinv for the transposed layer-1 epilogue
                    for b in range(nblk):
                        pdv = psB.tile([128, 128], F32, tag="aux")
                        nc.tensor.matmul(
                            pdv[:], lhsT=ones_sb[:],
                            rhs=deg_rsb[:, b * BLK : (b + 1) * BLK],
                            start=True, stop=True,
                        )
                        nc.vector.tensor_copy(
                            dinv_bc[:, b * BLK : (b + 1) * BLK], pdv[:])
                    nc.scalar.sqrt(dinv_bc[:], dinv_bc[:])
                    nc.vector.reciprocal(dinv_bc[:], dinv_bc[:])

            with (
                tc.tile_pool(name="gath", bufs=16) as gathp,
            ):
                # zero the gather pool once: trailing-trimmed lanes expose
                # stale SBUF; first use must not contain NaN-decoding garbage
                for zi in range(16):
                    zt = gathp.tile([128, MAXCH, HID], BF16, tag="g",
                                    name=f"z{zi}")
                    nc.vector.memset(zt[:, :, :], 0.0)

                def issue_gather(pool, tag, table, c0, sch, elem):
                    t = pool.tile([128, MAXCH, HID], BF16, tag=tag,
                                  name=f"gt{c0}")
                    nc.gpsimd.dma_gather(
                        t[:, :sch, :],
                        table.ap(),
                        gidx_sb[:, c0 * 8 : (c0 + sch) * 8],
                        sch * 128,
                        segv[c0],
                        elem,
                        queue_num=next_q(),
                        single_packet=True,
                    )
                    return t

                def one_mm(layer, acc, b, t, cc, gc, stop):
                    w = widths[b]
                    if layer == 1:
                        nc.tensor.matmul(
                            acc[b][:, :w],
                            lhsT=t[:, cc, :],
                            rhs=ohs_sb[:, gc * 128 : gc * 128 + w],
                            start=False,
                            stop=stop,
                        )
                    else:
                        nc.tensor.matmul(
                            acc[b][:w, :OUT],
                            lhsT=ohs_sb[:, gc * 128 : gc * 128 + w],
                            rhs=t[:, cc, :OUT],
                            start=False,
                            stop=stop,
                        )

                def run_layer(layer, stage_t, full_t, elem, acc_shape,
                              acc_pool, selfloop, epilogue, collective):
                    # the AllGather runs exclusively (concurrent SWDGE gather
                    # traffic throttles the CC DMA ~5x)
                    collective()
                    rem_tiles = {}
                    ri = [0]
                    LOOKAHEAD = 4

                    def issue_upto(n):
                        while ri[0] < min(n, len(rem_segs)):
                            c0, sch = rem_segs[ri[0]]
                            rem_tiles[ri[0]] = issue_gather(
                                gathp, "g", full_t, c0, sch, elem)
                            ri[0] += 1

                    # fine-grained block-major pipeline: per-block epilogues
                    # flow continuously; gathers stay LOOKAHEAD segs ahead
                    acc_cur = {}
                    for b in range(nblk):
                        issue_upto((cbase[b, 0] + int(CH[b, 0])) // MAXCH
                                   + LOOKAHEAD)
                        acc_cur[b] = acc_pool.tile(
                            acc_shape, F32, tag=f"acc{b % 4}",
                            name=f"acc{layer}_{b}",
                        )
                        selfloop(acc_cur, b)
                        for gc in range(cbase[b, 0], cbase[b, 0] + int(CH[b, 0])):
                            si = gc // MAXCH
                            t = rem_tiles[si]
                            one_mm(layer, acc_cur, b, t, gc - si * MAXCH, gc,
                                   gc == lastc[b])
                        epilogue(acc_cur, b)
                    issue_upto(len(rem_segs))

                # ---- layer 1: S1^T -> hsr^T -> hsr2 ----
                hsr2_big = constp.tile([128, nblk, 128], BF16, tag="hsr2big")
                nc.vector.memset(hsr2_big[:, :, OUT:], 0.0)
                with tc.tile_pool(name="psD", bufs=1, space="PSUM") as psD:
                    p2s_bufs = [
                        psD.tile([128, 128], F32, tag=f"p2s{i}", name=f"p2s{i}")
                        for i in range(2)
                    ]
                    epi_i = [0]

                    def selfloop1(acc, b):
                        w = widths[b]
                        nc.tensor.matmul(
                            acc[b][:, :w], lhsT=hs1_big[:w, b, :],
                            rhs=ident_sb[:w, :w],
                            start=True, stop=False,
                        )

                    def epilogue1(acc, b):
                        w = widths[b]
                        t1 = hsp.tile([128, 128], F32, tag="t1", name=f"t1_{b}")
                        nc.vector.tensor_tensor(
                            out=t1[:, :w], in0=acc[b][:, :w],
                            in1=dinv_bc[:, b * BLK : b * BLK + w],
                            op=mybir.AluOpType.mult,
                        )
                        hsrT = hsp.tile([128, 128], F32, tag="hsrT",
                                        name=f"hsrT_{b}")
                        nc.scalar.activation(
                            hsrT[:, :w], t1[:, :w],
                            mybir.ActivationFunctionType.Relu,
                            bias=b1_sb[:, 0:1],
                        )
                        p2s = p2s_bufs[epi_i[0] % 2]
                        epi_i[0] += 1
                        nc.tensor.matmul(
                            p2s[:w, :OUT], lhsT=hsrT[:, :w], rhs=w2_sb[:, :],
                            start=True, stop=True,
                        )
                        nc.scalar.activation(
                            hsr2_big[:w, b, :OUT], p2s[:w, :OUT],
                            mybir.ActivationFunctionType.Copy,
                            scale=dinv_sb[:w, b : b + 1],
                        )

                    def ag1():
                        nc.gpsimd.collective_compute(
                            "AllGather",
                            mybir.AluOpType.bypass,
                            replica_groups=rg,
                            ins=[hs1_stage[0:NPS, :].opt()],
                            outs=[hs1_full[0 : ncores * NPS, :].opt()],
                        )

                    run_layer(1, hs1_stage, hs1_full, HID, [128, 128], psD,
                              selfloop1, epilogue1, ag1)
                    nc.scalar.dma_start(
                        out=hsr2_stage[0:NPS, :], in_=hsr2_big[:, :, :],
                    )

                # ---- layer 2: S2 -> y ----
                with tc.tile_pool(name="psF", bufs=1, space="PSUM") as psF:

                    def selfloop2(acc, b):
                        w = widths[b]
                        nc.tensor.matmul(
                            acc[b][:w, :OUT], lhsT=ident_sb[:w, :w],
                            rhs=hsr2_big[:w, b, :OUT],
                            start=True, stop=False,
                        )

                    def epilogue2(acc, b):
                        w = widths[b]
                        o1 = hsp.tile([128, OUT], F32, tag="o1", name=f"o1_{b}")
                        nc.scalar.activation(
                            o1[:w, :], acc[b][:w, :OUT],
                            mybir.ActivationFunctionType.Copy,
                            scale=dinv_sb[:w, b : b + 1],
                        )
                        yt = hsp.tile([128, OUT], F32, tag="yt", name=f"yt_{b}")
                        nc.vector.tensor_tensor(
                            out=yt[:w, :], in0=o1[:w, :], in1=b2_bc[:w, :],
                            op=mybir.AluOpType.add,
                        )
                        nc.scalar.dma_start(out=y[b * BLK : b * BLK + w, :],
                                              in_=yt[:w, :])

                    def ag2():
                        nc.gpsimd.collective_compute(
                            "AllGather",
                            mybir.AluOpType.bypass,
                            replica_groups=rg,
                            ins=[hsr2_stage[0:NPS, :].opt()],
                            outs=[hsr2_full[0 : ncores * NPS, :].opt()],
                        )

                    run_layer(2, hsr2_stage, hsr2_full, 128, [128, 64], psF,
                              selfloop2, epilogue2, ag2)

    nc.compile()
    return nc


def _make_ident():
    import ml_dtypes

    return np.eye(128, dtype=np.float32).astype(ml_dtypes.bfloat16)


_IDENT = _make_ident()


def make_in_maps(per_core, W1, b1, W2, b2):
    import ml_dtypes

    W1 = np.ascontiguousarray(np.asarray(W1, np.float32).astype(ml_dtypes.bfloat16))
    W2 = np.ascontiguousarray(np.asarray(W2, np.float32))
    b1 = np.ascontiguousarray(np.asarray(b1, np.float32).reshape(-1, 1))
    b2 = np.asarray(b2, np.float32).reshape(1, -1)
    return [
        {
            "x_tr": pc["x_tr"],
            "w1": W1,
            "b1c": b1,
            "w2": W2,
            "b2": b2,
            "deg_own": pc["deg_own"],
            "deg_row": pc["deg_row"],
            "gidx": pc["gidx"],
            "ohs": pc["ohs"],
            "ident": _IDENT,
        }
        for pc in per_core
    ]


def kernel_run(x, edge_index, W1, b1, W2, b2, trace=False, tmpdir=None):
    x = np.ascontiguousarray(np.asarray(x, np.float32))
    per_core, meta = preprocess(x, edge_index)
    HID = np.asarray(W1).shape[1]
    OUT = np.asarray(W2).shape[1]
    nc = build_nc(meta, HID, OUT)
    in_maps = make_in_maps(per_core, W1, b1, W2, b2)
    res = run_bass_kernel_spmd(
        nc, in_maps, core_ids=list(range(NCORES)), trace=trace, tmpdir=tmpdir
    )
    out = np.concatenate([r["y"] for r in res.results], axis=0)
    return out, res


def kernel(x, edge_index, W1, b1, W2, b2):
    out, _ = kernel_run(x, edge_index, W1, b1, W2, b2)
    return out
